# revision 44
# baseline (speedup 1.0000x reference)
"""Trainium2 Bass kernel for cumulative-state (linear) attention over M modalities.

Math (reference): out[i, e] = sum_m sum_{j : t2_m[j] <= t1[i]} (Q[i] . K_m[j]) * X_m[j, e],
for e in {0, 1}, where Q = mlp_q(X[0]), K_m = mlp_km(X[m]), t1 = X[0,:,-1], t2_m = X[m,:,-1].

Sharding: 8 cores = (m, h): modality m in 0..3, key-half h in 0..1. Each core owns
keys j in [h*4096, (h+1)*4096) of modality m and computes partial contributions for
ALL queries; the host scatter-sums the 8 partial outputs (the "all-reduce").

Per core the 4096 keys are split into NKC=33 chunks of CK=126 keys (last chunk 64).
Each chunk occupies a 128-column slot in the packed key layout: 126 key columns plus
2 reserved columns that are later overwritten with the chunk's running prefix state
srun_k (64, 2). Query i with idx in chunk k gets its full contribution from ONE
masked matmul pair:
  B = [K_chunk | srun_k]^T q          (pB: 128 rows = 126 keys + 2 state rows)
  out = [V2_chunk ; I2]^T (B * mask)  (po: mask rows 126,127 are all-ones)
so the state term and the intra-chunk causal term come out of a single accumulation.

The chunk states S_k = K_k^T V2_k are computed as 5 grouped matmuls (8 chunks per
matmul, diagonal blocks extracted), prefix-summed with ONE lower-triangular constant
matmul (no serial DVE chain), and transposed back via an identity matmul. Everything
64-contract is row-packed two-sides-per-128-partitions with concurrent quadrant
matmul pairs. Epilogues and copies are spread over ACT/DVE/GPSIMD.
"""

import os
from contextlib import ExitStack

import ml_dtypes
import numpy as np

BF16 = ml_dtypes.bfloat16

M, T, D = 4, 8192, 64
NLIN = 3
CK = 126         # keys per chunk
CH = 128         # chunk column stride (126 keys + 2 srun slots)
NK = T // 2      # keys per core (4096)
NKC = 33         # chunks per core (32*126 + 64)
NCORES = 8
FMAX = 512       # max matmul free dim / PSUM bank cols (f32)
NSMAX = 17       # max chunks per side
KW = NSMAX * CH  # packed key cols per side (2176)
NG = 4           # psc groups of 8 chunks (+1 straggler chunk 32)
NR = 2 * (NKC + 1)  # srunT rows (68)


def _round_up(x, k):
    return ((x + k - 1) // k) * k


def _scrow(k):
    """scT row for chunk k (gapped layout from tile_position col offsets)."""
    g, i = divmod(k, 8)
    return 32 * g + 2 * i


def make_plan(X):
    """Host-side: band structure + packed column layout, shared across cores."""
    X = np.asarray(X, np.float32)
    t1 = X[0, :, -1]
    los, his, tbs, idxs = [], [], [], []
    for c in range(NCORES):
        m, h = c // 2, c % 2
        t2 = X[m, :, -1]
        idx = np.searchsorted(t2, t1, side="right") - 1
        idxs.append(idx)
        hs = h * NK
        edges = [hs + min(k * CK, NK) for k in range(NKC + 1)]
        lo = np.searchsorted(idx, edges[:-1], side="left")
        hi = np.searchsorted(idx, edges[1:], side="left")
        los.append(lo)
        his.append(hi)
        tbs.append(int(np.searchsorted(idx, hs + NK, side="left")))

    NB = [0] * NKC
    for k in range(NKC):
        w = max(his[c][k] - los[c][k] for c in range(NCORES))
        NB[k] = _round_up(int(w), 8)

    # split chunks into two sides (partition halves) with balanced band totals
    order = sorted(range(NKC), key=lambda k: -NB[k])
    sideof = [0] * NKC
    tot = [0, 0]
    cnt = [0, 0]
    for k in order:
        s = 0 if (tot[0] <= tot[1] and cnt[0] < NSMAX) or cnt[1] >= NSMAX else 1
        sideof[k] = s
        tot[s] += NB[k]
        cnt[s] += 1
    TOFF = _round_up(max(tot[0], tot[1]), 8)

    kpos = [0] * NKC
    qoff = [0] * NKC
    acc = [0, 0]
    pos = [0, 0]
    for k in range(NKC):  # global ascending within each side
        s = sideof[k]
        kpos[k] = pos[s]
        qoff[k] = acc[s]
        pos[s] += 1
        acc[s] += NB[k]

    NT = _round_up(max(T - tb for tb in tbs), 8)
    tl0 = min(_round_up((NT + 1) // 2, 8), NT)
    tlen = [tl0, NT - tl0]
    NW2 = TOFF + max(tlen)
    korder = sorted(range(NKC), key=lambda k: (sideof[k], qoff[k]))

    return dict(NB=NB, TOFF=TOFF, NW2=NW2, sideof=sideof, kpos=kpos, qoff=qoff,
                tlen=tlen, los=los, his=his, tbs=tbs, idxs=idxs, korder=korder,
                lb=tot[0], rb=tot[1])


def make_inputs(X, wq_w, wq_b, wk_w, wk_b, plan):
    X = np.asarray(X, np.float32)
    wq_w = np.asarray(wq_w, np.float32)
    wq_b = np.asarray(wq_b, np.float32)
    wk_w = np.asarray(wk_w, np.float32)
    wk_b = np.asarray(wk_b, np.float32)
    NB, TOFF, NW2 = plan["NB"], plan["TOFF"], plan["NW2"]
    sideof, kpos, qoff = plan["sideof"], plan["kpos"], plan["qoff"]
    tlen = plan["tlen"]

    # weights stacked into both partition halves
    wq1 = np.concatenate([wq_w[l] for l in range(NLIN)], axis=1)
    wq = np.concatenate([wq1, wq1], axis=0).astype(BF16)              # (128, 192)
    bq1 = np.stack([wq_b[l] for l in range(NLIN)], axis=1)
    bq = np.concatenate([bq1, bq1], axis=0).astype(np.float32)        # (128, 3)

    # prefix-sum matrices, one per within-group chunk position i: row
    # 32g+2i+e of stripe i contributes chunk 8g+i's S to srunT[2k'+e] for
    # k' > 8g+i. Zero rows mask the off-diagonal garbage in the stripes.
    lmatA = np.zeros((128, 8 * NR), np.float32)
    lmatB = np.zeros((2, NR), np.float32)
    for k in range(32):
        g, i = divmod(k, 8)
        for kp in range(k + 1, NKC + 1):
            for e in range(2):
                lmatA[32 * g + 2 * i + e, i * NR + 2 * kp + e] = 1.0
    for kp in range(33, NKC + 1):
        lmatB[0, 2 * kp] = 1.0
        lmatB[1, 2 * kp + 1] = 1.0
    lmatA = lmatA.astype(BF16)
    lmatB = lmatB.astype(BF16)
    # transpose-permutation: pt col j holds srun of the j-th chunk in
    # (side, kpos) order, so the kt injection is one strided copy per side
    ns0 = sum(1 for s in sideof if s == 0)
    p68 = np.zeros((NR, NR), BF16)
    for k in range(NKC):
        j0 = (0 if sideof[k] == 0 else 2 * ns0) + 2 * kpos[k]
        p68[2 * k, j0] = 1.0
        p68[2 * k + 1, j0 + 1] = 1.0
    p68[2 * NKC, 2 * NKC] = 1.0
    p68[2 * NKC + 1, 2 * NKC + 1] = 1.0

    in_maps = []
    for c in range(NCORES):
        m, h = c // 2, c % 2
        hs = h * NK
        lo, hi, tb = plan["los"][c], plan["his"][c], plan["tbs"][c]
        idx = plan["idxs"][c]

        qb = np.zeros((2 * D, NW2), BF16)
        msk = np.zeros((CH, 2 * TOFF), BF16)
        for k in range(NKC):
            n = hi[k] - lo[k]
            s, o = sideof[k], qoff[k]
            if n > 0:
                qb[64 * s:64 * s + 64, o:o + n] = X[0, lo[k]:hi[k], :].T.astype(BF16)
                jg = hs + k * CK + np.arange(CK)[:, None]
                msk[0:CK, s * TOFF + o:s * TOFF + o + n] = \
                    (jg <= idx[None, lo[k]:hi[k]]).astype(BF16)
                msk[CK:CH, s * TOFF + o:s * TOFF + o + n] = 1.0
        # tail: first tlen[0] tail queries on side 0, rest on side 1
        ntail = T - tb
        n0 = min(ntail, tlen[0])
        if n0 > 0:
            qb[0:64, TOFF:TOFF + n0] = X[0, tb:tb + n0, :].T.astype(BF16)
        n1 = ntail - n0
        if n1 > 0:
            qb[64:128, TOFF:TOFF + n1] = X[0, tb + n0:, :].T.astype(BF16)

        xk = X[m, hs:hs + NK, :]
        xkt = np.zeros((2 * D, KW), BF16)
        v2 = np.zeros((CH, 2 * NKC), BF16)
        for k in range(NKC):
            s, p = sideof[k], kpos[k]
            a, b = k * CK, min((k + 1) * CK, NK)
            nk = b - a
            xkt[64 * s:64 * s + 64, p * CH:p * CH + nk] = xk[a:b, :].T.astype(BF16)
            v2[0:nk, 2 * k:2 * k + 2] = xk[a:b, 0:2].astype(BF16)
            v2[CK, 2 * k] = 1.0      # I2 rows: pass srun rows of bm through po
            v2[CK + 1, 2 * k + 1] = 1.0

        wk1 = np.concatenate([wk_w[m, l] for l in range(NLIN)], axis=1)
        wk = np.concatenate([wk1, wk1], axis=0).astype(BF16)          # (128, 192)
        bk1 = np.stack([wk_b[m, l] for l in range(NLIN)], axis=1)
        bk = np.concatenate([bk1, bk1], axis=0).astype(np.float32)    # (128, 3)

        # host-computed S correction: the last K-linear's bias contributes
        # b3 (x) sum_j v2[j,:] per chunk — prefix-accumulated on host and
        # added once to srunT after the triangular matmul
        b3 = wk_b[m, NLIN - 1]                                        # (64,)
        stc = np.zeros((NR, D), np.float32)
        acc = np.zeros((2, D), np.float32)
        for k in range(NKC + 1):
            stc[2 * k] = acc[0]
            stc[2 * k + 1] = acc[1]
            if k < NKC:
                a, b = k * CK, min((k + 1) * CK, NK)
                vs = xk[a:b, 0:2].astype(BF16).astype(np.float32).sum(axis=0)
                acc[0] += b3 * vs[0]
                acc[1] += b3 * vs[1]

        in_maps.append(dict(qb=qb, msk=msk, xkt=xkt, v2=v2,
                            wq=wq, bq=bq, wk=wk, bk=bk, stc=stc,
                            lmatA=lmatA, lmatB=lmatB, p68=p68))
    return in_maps


def scatter_outputs(plan, outs):
    """Host-side 'all-reduce': scatter per-core (4, NW2) partials to (T, 2)."""
    NB, TOFF = plan["NB"], plan["TOFF"]
    sideof, qoff, tlen = plan["sideof"], plan["qoff"], plan["tlen"]
    y = np.zeros((T, 2), np.float32)
    for c in range(NCORES):
        o4 = np.asarray(outs[c], np.float32)
        lo, hi, tb = plan["los"][c], plan["his"][c], plan["tbs"][c]
        for k in range(NKC):
            n = hi[k] - lo[k]
            if n > 0:
                s, o = sideof[k], qoff[k]
                y[lo[k]:hi[k], :] += o4[2 * s:2 * s + 2, o:o + n].T
        ntail = T - tb
        n0 = min(ntail, tlen[0])
        if n0 > 0:
            y[tb:tb + n0, :] += o4[0:2, TOFF:TOFF + n0].T
        n1 = ntail - n0
        if n1 > 0:
            y[tb + n0:, :] += o4[2:4, TOFF:TOFF + n1].T
    return y


# ---------------------------------------------------------------- numpy emulation
def emulate_core(im, plan):
    """Numpy mirror of the device graph for one core (f32 math, for validation)."""
    NB, TOFF, NW2 = plan["NB"], plan["TOFF"], plan["NW2"]
    sideof, kpos, qoff = plan["sideof"], plan["kpos"], plan["qoff"]
    tlen = plan["tlen"]

    def f(x):
        return np.asarray(x, np.float32)

    wk, bk, wq, bq = f(im["wk"]), f(im["bk"]), f(im["wq"]), f(im["bq"])
    qb, xkt, v2, msk = f(im["qb"]), f(im["xkt"]), f(im["v2"]), f(im["msk"])
    lmatA, lmatB = f(im["lmatA"]), f(im["lmatB"])

    def mlp_packed(xp, w, b):
        a = xp
        outs = []
        for l in range(NLIN):
            z = np.concatenate([
                w[0:64, 64 * l:64 * (l + 1)].T @ a[0:64] + b[0:64, l][:, None],
                w[64:128, 64 * l:64 * (l + 1)].T @ a[64:128] + b[64:128, l][:, None],
            ], axis=0)
            a = np.maximum(z, 0.0) if l < NLIN - 1 else z
            outs.append(a)
        return outs[-1], outs[-2]

    ktp, a2p = mlp_packed(xkt, wk, bk)       # (128, KW)
    qtp, _ = mlp_packed(qb, wq, bq)          # (128, NW2)

    # kn (keys-on-partitions, bias-free), per chunk
    kn = np.zeros((CK, D * NKC), np.float32)
    for k in range(NKC):
        s, p = sideof[k], kpos[k]
        a2 = a2p[64 * s:64 * s + 64, p * CH:p * CH + CK]
        kn[:, D * k:D * (k + 1)] = a2.T @ wk[64 * s:64 * s + 64, 2 * D:3 * D]

    # grouped psc stripes (diag blocks live at rows 32g+2i, cols 64i)
    sct1 = np.zeros((128, FMAX), np.float32)
    scTB = np.zeros((2, D), np.float32)
    for k in range(NKC):
        blk = v2[0:CK, 2 * k:2 * k + 2].T @ kn[:, D * k:D * (k + 1)]  # (2, 64)
        if k < 32:
            g, i = divmod(k, 8)
            sct1[32 * g + 2 * i:32 * g + 2 * i + 2, D * i:D * (i + 1)] = blk
        else:
            scTB = blk
    srunT = f(im["stc"]).copy()                             # (68, 64)
    for i in range(8):
        srunT += lmatA[:, i * NR:(i + 1) * NR].T @ sct1[:, D * i:D * (i + 1)]
    srunT += lmatB.T @ scTB
    srunP = srunT.T @ f(im["p68"])                          # (64, 68) side-ordered

    # inject srun into the 2 reserved kt columns of each chunk (strided)
    ns0 = sum(1 for s in sideof if s == 0)
    for k in range(NKC):
        s, p = sideof[k], kpos[k]
        j0 = (0 if s == 0 else 2 * ns0) + 2 * p
        ktp[64 * s:64 * s + 64, p * CH + CK:p * CH + CH] = srunP[:, j0:j0 + 2]
    srun_tot = srunP[:, 2 * NKC:2 * NKC + 2]

    out = np.zeros((66, NW2), np.float32)
    # tails (total-state term)
    for s in range(2):
        n = tlen[s]
        if n > 0:
            out[64 * s:64 * s + 2, TOFF:TOFF + n] = \
                srun_tot.T @ qtp[64 * s:64 * s + 64, TOFF:TOFF + n]
    # band chunks: one masked matmul pair per chunk
    for k in range(NKC):
        nq = NB[k]
        if nq == 0:
            continue
        s, p, o = sideof[k], kpos[k], qoff[k]
        qblk = qtp[64 * s:64 * s + 64, o:o + nq]
        B = ktp[64 * s:64 * s + 64, p * CH:(p + 1) * CH].T @ qblk    # (128, nq)
        bm = B * msk[:, s * TOFF + o:s * TOFF + o + nq]
        out[64 * s:64 * s + 2, o:o + nq] = v2[:, 2 * k:2 * k + 2].T @ bm
    return np.concatenate([out[0:2], out[64:66]], axis=0)  # (4, NW2)


# ---------------------------------------------------------------- device graph
def build_graph(plan):
    import concourse.bacc as bacc
    import concourse.tile as tile
    from concourse import mybir

    NB, TOFF, NW2 = plan["NB"], plan["TOFF"], plan["NW2"]
    sideof, kpos, qoff = plan["sideof"], plan["kpos"], plan["qoff"]
    tlen, korder = plan["tlen"], plan["korder"]
    f32 = mybir.dt.float32
    bf16 = mybir.dt.bfloat16
    AF = mybir.ActivationFunctionType
    OP = mybir.AluOpType

    nc = bacc.Bacc("TRN2")
    d_qb = nc.dram_tensor("qb", [2 * D, NW2], bf16, kind="ExternalInput")
    d_msk = nc.dram_tensor("msk", [CH, 2 * TOFF], bf16, kind="ExternalInput")
    d_xkt = nc.dram_tensor("xkt", [2 * D, KW], bf16, kind="ExternalInput")
    d_v2 = nc.dram_tensor("v2", [CH, 2 * NKC], bf16, kind="ExternalInput")
    d_wq = nc.dram_tensor("wq", [2 * D, D * NLIN], bf16, kind="ExternalInput")
    d_bq = nc.dram_tensor("bq", [2 * D, NLIN], f32, kind="ExternalInput")
    d_wk = nc.dram_tensor("wk", [2 * D, D * NLIN], bf16, kind="ExternalInput")
    d_bk = nc.dram_tensor("bk", [2 * D, NLIN], f32, kind="ExternalInput")
    d_stc = nc.dram_tensor("stc", [NR, D], f32, kind="ExternalInput")
    d_lA = nc.dram_tensor("lmatA", [128, 8 * NR], bf16, kind="ExternalInput")
    d_lB = nc.dram_tensor("lmatB", [2, NR], bf16, kind="ExternalInput")
    d_p68 = nc.dram_tensor("p68", [NR, NR], bf16, kind="ExternalInput")
    ns0 = sum(1 for s in sideof if s == 0)
    ns1 = NKC - ns0
    d_out = nc.dram_tensor("out", [4, NW2], f32, kind="ExternalOutput")

    with ExitStack() as ctx:
        tc = ctx.enter_context(tile.TileContext(nc))
        const = ctx.enter_context(tc.tile_pool(name="const", bufs=1))
        big = ctx.enter_context(tc.tile_pool(name="big", bufs=1))
        work = ctx.enter_context(tc.tile_pool(name="work", bufs=3))
        pmlp = ctx.enter_context(tc.tile_pool(name="pmlp", bufs=3, space="PSUM"))
        pb = ctx.enter_context(tc.tile_pool(name="pb", bufs=2, space="PSUM"))
        ps = ctx.enter_context(tc.tile_pool(name="ps", bufs=1, space="PSUM"))
        pout = ctx.enter_context(tc.tile_pool(name="pout", bufs=2, space="PSUM"))

        wq_t = const.tile([2 * D, D * NLIN], bf16, tag="wq")
        bq_t = const.tile([2 * D, NLIN], f32, tag="bq")
        wk_t = const.tile([2 * D, D * NLIN], bf16, tag="wk")
        bk_t = const.tile([2 * D, NLIN], f32, tag="bk")
        stc_t = const.tile([NR, D], f32, tag="stc")
        lA_t = const.tile([128, 8 * NR], bf16, tag="lA")
        lB_t = const.tile([2, NR], bf16, tag="lB")
        p68_t = const.tile([NR, NR], bf16, tag="p68")
        v2_t = const.tile([CH, 2 * NKC], bf16, tag="v2")

        # input DMA triggers live on SP + GPSIMD queues ONLY: a trigger stalls
        # its queue head until a hardware DMA slot frees, and ACT/DVE must not
        # be blocked behind that (they run the MLP epilogues)
        nc.sync.dma_start(wk_t[:], d_wk[:])
        xkt_t = big.tile([2 * D, KW], bf16, tag="xkt")
        for i in range(4):
            a, b = i * (KW // 4), (i + 1) * (KW // 4)
            nc.sync.dma_start(xkt_t[:, a:b], d_xkt[:, a:b])
        nc.sync.dma_start(bk_t[:], d_bk[:])
        nc.sync.dma_start(v2_t[:], d_v2[:])
        nc.sync.dma_start(stc_t[:], d_stc[:])
        nc.sync.dma_start(lA_t[:], d_lA[:])
        nc.sync.dma_start(lB_t[:], d_lB[:])
        nc.sync.dma_start(p68_t[:], d_p68[:])
        nc.sync.dma_start(wq_t[:], d_wq[:])
        nc.sync.dma_start(bq_t[:], d_bq[:])
        qb_t = big.tile([2 * D, NW2], bf16, tag="qb")
        msk_t = big.tile([CH, 2 * TOFF], bf16, tag="msk")

        kt_t = big.tile([2 * D, KW], bf16, tag="kt")
        a2k_t = big.tile([2 * D, KW], bf16, tag="a2k")
        qt_t = big.tile([2 * D, NW2], bf16, tag="qt")
        kn_t = big.tile([CH, D * NKC], bf16, tag="kn")
        sct1_t = big.tile([128, FMAX], bf16, tag="sct1")
        scTB_t = big.tile([2, D], bf16, tag="scTB")
        srunT_t = big.tile([NR, D], bf16, tag="srunT")
        st_t = big.tile([2 * D, 2], bf16, tag="st")
        outs_t = big.tile([66, NW2], f32, tag="outs")

        # engine cycling helper: PSUM-reading copies alternate ACT/DVE
        # (GPSIMD cannot access PSUM)
        cp_state = [0]
        def cyc_copy(dst, src):
            i = cp_state[0] = (cp_state[0] + 1) % 2
            if i == 0:
                nc.scalar.copy(dst, src)
            else:
                nc.vector.tensor_copy(dst, src)

        # PE warm-up burst: dense dummy matmuls during the initial DMA window
        # flip the HAM clock gate / p-state before real work arrives
        wup_t = work.tile([CH, FMAX], bf16, tag="wup", name="wup")
        nc.vector.memset(wup_t[:], 0.0)
        nc.gpsimd.memset(sct1_t[:], 0.0)
        dma_eng = {"sync": nc.sync, "scalar": nc.scalar,
                   "gpsimd": nc.gpsimd}[os.environ.get("KDMA", "gpsimd")]
        NQB = 4
        qsp = _round_up((NW2 + NQB - 1) // NQB, 8)
        for i in range(NQB):
            a, b = i * qsp, min((i + 1) * qsp, NW2)
            if a < b:
                dma_eng.dma_start(qb_t[:, a:b], d_qb[:, a:b])
        NMQ = 4
        msp = _round_up((2 * TOFF + NMQ - 1) // NMQ, 8)
        for i in range(NMQ):
            a, b = i * msp, min((i + 1) * msp, 2 * TOFF)
            if a < b:
                dma_eng.dma_start(msk_t[:, a:b], d_msk[:, a:b])
        for _ in range(int(os.environ.get("KWUP", "4"))):
            pwu = pb.tile([CH, FMAX], f32, tag="pb", name="pwu")
            nc.tensor.matmul(pwu[:], wup_t[:, 0:CH], wup_t[:], start=True, stop=True)

        ep_state = [0]

        def emit_layer(l, src_t, dst_t, w_t, b_t, n_cols):
            """One row-packed MLP layer: stream all column blocks through the
            PE back-to-back (same stationary weights), bias+relu epilogues
            alternating ACT/DVE (ACT is faster: give it 2 of 3)."""
            for a in range(0, n_cols, FMAX):
                b = min(a + FMAX, n_cols)
                n = b - a
                cur = src_t[:, a:b]
                pz = pmlp.tile([CH, FMAX], f32, tag="pmlp", name="pz")
                nc.tensor.matmul(pz[0:64, :n], w_t[0:64, D * l:D * (l + 1)],
                                 cur[0:64, :], start=True, stop=True,
                                 tile_position=(0, 0))
                nc.tensor.matmul(pz[64:128, :n], w_t[64:128, D * l:D * (l + 1)],
                                 cur[64:128, :], start=True, stop=True,
                                 tile_position=(64, 64))
                dst = dst_t[:, a:b]
                e = ep_state[0] = (ep_state[0] + 1) % 3
                if e == 1:
                    if l < NLIN - 1:
                        nc.vector.tensor_scalar(dst, pz[:, :n],
                                                b_t[:, l:l + 1], 0.0,
                                                OP.add, OP.max)
                    else:
                        nc.vector.tensor_scalar_add(dst, pz[:, :n],
                                                    b_t[:, l:l + 1])
                else:
                    nc.scalar.activation(dst, pz[:, :n],
                                         AF.Relu if l < NLIN - 1 else AF.Identity,
                                         bias=b_t[:, l:l + 1])

        # K mlp
        mlpk_t = work.tile([CH, KW], bf16, tag="mlpa", name="mlpk")
        emit_layer(0, xkt_t, mlpk_t, wk_t, bk_t, KW)
        emit_layer(1, mlpk_t, a2k_t, wk_t, bk_t, KW)
        emit_layer(2, a2k_t, kt_t, wk_t, bk_t, KW)

        # kn: K natural per chunk (bias-free; bias folded into stc), two
        # chunks per PSUM tile so the drain copies halve
        if os.environ.get("KKNP", "0") != "0":
            for k0 in range(0, NKC, 2):
                pkn = pb.tile([CH, FMAX], f32, tag="pb", name="pkn")
                for k in (k0, k0 + 1):
                    if k >= NKC:
                        continue
                    s, p = sideof[k], kpos[k]
                    nc.tensor.matmul(pkn[0:CK, D * (k - k0):D * (k - k0 + 1)],
                                     a2k_t[64 * s:64 * s + 64, CH * p:CH * p + CK],
                                     wk_t[64 * s:64 * s + 64, 2 * D:3 * D],
                                     start=True, stop=True,
                                     tile_position=(64 * s, 0))
                n = D * min(2, NKC - k0)
                cyc_copy(kn_t[0:CK, D * k0:D * k0 + n], pkn[0:CK, :n])
        else:
            for k in range(NKC):
                s, p = sideof[k], kpos[k]
                pkn = pb.tile([CH, FMAX], f32, tag="pb", name="pkn")
                nc.tensor.matmul(pkn[0:CK, :D],
                                 a2k_t[64 * s:64 * s + 64, CH * p:CH * p + CK],
                                 wk_t[64 * s:64 * s + 64, 2 * D:3 * D],
                                 start=True, stop=True, tile_position=(64 * s, 0))
                cyc_copy(kn_t[0:CK, D * k:D * (k + 1)], pkn[0:CK, :D])

        # grouped S chunks: 4 matmuls of 8 chunks (diag blocks) + straggler
        psc = ps.tile([128, FMAX], f32, tag="ps", name="psc")
        for g in range(NG):
            nc.tensor.matmul(psc[32 * g:32 * g + 16, 0:FMAX],
                             v2_t[0:CK, 16 * g:16 * g + 16],
                             kn_t[0:CK, FMAX * g:FMAX * (g + 1)],
                             start=True, stop=True, tile_position=(0, 32 * g))
        psB = pb.tile([CH, FMAX], f32, tag="pb", name="psB")
        nc.tensor.matmul(psB[0:2, :D], v2_t[0:CK, 64:66],
                         kn_t[0:CK, D * 32:D * 33],
                         start=True, stop=True, tile_position=(0, 0))
        # stripe drain: PSUM group stripes -> SBUF bf16 (32-aligned partition
        # starts; finer PSUM reads are illegal). Gap rows were memset once so
        # the prefix matmuls' zero weights never hit NaN garbage.
        for g in range(NG):
            cyc_copy(sct1_t[32 * g:32 * g + 16, :], psc[32 * g:32 * g + 16, :])
        cyc_copy(scTB_t[:], psB[0:2, :D])

        # Q mlp layer 1 keeps the PE busy across the srun dependency chain
        mlpq_t = work.tile([CH, NW2], bf16, tag="mlpa", name="mlpq")
        emit_layer(0, qb_t, mlpq_t, wq_t, bq_t, NW2)

        # prefix sums via 9 accumulating triangular matmuls straight off the
        # stripes (lmatA_i's zero rows mask the off-diagonal stripe garbage),
        # then the host bias-prefix correction is added in the PSUM drain
        pl = pb.tile([CH, FMAX], f32, tag="pb", name="pl")
        for i in range(8):
            nc.tensor.matmul(pl[0:NR, :D], lA_t[:, NR * i:NR * (i + 1)],
                             sct1_t[:, D * i:D * (i + 1)],
                             start=(i == 0), stop=False, tile_position=(0, 0))
        nc.tensor.matmul(pl[0:NR, :D], lB_t[:], scTB_t[:], start=False, stop=True,
                         tile_position=(0, 0))
        nc.vector.tensor_add(srunT_t[:], pl[0:NR, :D], stc_t[:])

        # transpose back via the side-order permutation matmul, duplicated
        # into both halves; srun lands pre-sorted for one strided copy/side
        pt = pb.tile([CH, FMAX], f32, tag="pb", name="pt")
        nc.tensor.matmul(pt[0:64, 0:NR], srunT_t[:], p68_t[:],
                         start=True, stop=True, tile_position=(0, 0))
        nc.tensor.matmul(pt[64:128, 0:NR], srunT_t[:], p68_t[:],
                         start=True, stop=True, tile_position=(0, 64))
        # inject srun_k into every chunk's 2 reserved kt columns: one strided
        # copy per side (33 tiny copies serialized horribly here before)
        if os.environ.get("KINJ", "strided") == "strided":
            for s, nss, base in ((0, ns0, 0), (1, ns1, 2 * ns0)):
                dst = kt_t[64 * s:64 * s + 64, 0:KW].rearrange(
                    "p (c w) -> p c w", w=CH)[:, 0:nss, CK:CH]
                src = pt[64 * s:64 * s + 64, base:base + 2 * nss].rearrange(
                    "p (c two) -> p c two", two=2)
                if s == 0:
                    nc.scalar.copy(dst, src)
                else:
                    nc.vector.tensor_copy(dst, src)
        else:
            for k in range(NKC):
                s, p = sideof[k], kpos[k]
                j0 = (0 if s == 0 else 2 * ns0) + 2 * p
                cyc_copy(kt_t[64 * s:64 * s + 64, CH * p + CK:CH * p + CH],
                         pt[64 * s:64 * s + 64, j0:j0 + 2])
        # total state for the tails
        nc.scalar.copy(st_t[0:64, :], pt[0:64, 2 * NKC:2 * NKC + 2])
        nc.scalar.copy(st_t[64:128, :], pt[64:128, 2 * NKC:2 * NKC + 2])

        # Q mlp layers 2-3 (mid shares qt_t — Tile serializes per-block WAR)
        emit_layer(1, mlpq_t, qt_t, wq_t, bq_t, NW2)
        emit_layer(2, qt_t, qt_t, wq_t, bq_t, NW2)

        # tails (state-only), sides paired on PE quadrants
        for a in range(0, max(tlen), FMAX):
            for s in range(2):
                b = min(a + FMAX, tlen[s])
                if a >= b:
                    continue
                n = b - a
                po = pout.tile([CH, FMAX], f32, tag="pout", name="po_t")
                nc.tensor.matmul(po[64 * s:64 * s + 2, :n], st_t[64 * s:64 * s + 64, :],
                                 qt_t[64 * s:64 * s + 64, TOFF + a:TOFF + b],
                                 start=True, stop=True,
                                 tile_position=(64 * s, 64 * s))
                cyc_copy(outs_t[64 * s:64 * s + 2, TOFF + a:TOFF + b],
                         po[64 * s:64 * s + 2, :n])
        for s in range(2):
            if tlen[s] > 0:
                nc.sync.dma_start(d_out[2 * s:2 * s + 2, TOFF:TOFF + tlen[s]],
                                  outs_t[64 * s:64 * s + 2, TOFF:TOFF + tlen[s]])

        # band chunks: one masked matmul pair per chunk, software-pipelined
        # one deep so the PE never stalls on the DVE mask multiply
        segs = []
        for k in korder:
            nq = NB[k]
            s, p, o0 = sideof[k], kpos[k], int(qoff[k])
            for a in range(0, nq, FMAX):
                segs.append((k, s, p, o0 + a, min(a + FMAX, nq) - a))
        pBs = [None] * len(segs)

        def emit_pB(i):
            k, s, p, o, n = segs[i]
            pB = pBs[i] = pb.tile([CH, FMAX], f32, tag="pb", name="pB")
            nc.tensor.matmul(pB[:, :n], kt_t[64 * s:64 * s + 64, CH * p:CH * (p + 1)],
                             qt_t[64 * s:64 * s + 64, o:o + n],
                             start=True, stop=True, tile_position=(64 * s, 0))

        def emit_rest(i):
            k, s, p, o, n = segs[i]
            bm = work.tile([CH, FMAX], bf16, tag="bm", name="bm")
            nc.vector.tensor_mul(bm[:, :n], pBs[i][:, :n],
                                 msk_t[:, s * TOFF + o:s * TOFF + o + n])
            po = pout.tile([CH, FMAX], f32, tag="pout", name="po")
            nc.tensor.matmul(po[64 * s:64 * s + 2, :n], v2_t[:, 2 * k:2 * k + 2],
                             bm[:, :n], start=True, stop=True,
                             tile_position=(0, 64 * s))
            cyc_copy(outs_t[64 * s:64 * s + 2, o:o + n],
                     po[64 * s:64 * s + 2, :n])

        if os.environ.get("KSEQ", "0") != "0":
            for i in range(len(segs)):
                emit_pB(i)
                emit_rest(i)
        elif segs:
            emit_pB(0)
            for i in range(1, len(segs)):
                emit_pB(i)
                emit_rest(i - 1)
            emit_rest(len(segs) - 1)
        for s in range(2):
            w = plan["lb"] if s == 0 else plan["rb"]
            if w > 0:
                nc.sync.dma_start(d_out[2 * s:2 * s + 2, 0:w],
                                  outs_t[64 * s:64 * s + 2, 0:w])

    nc.finalize()
    return nc


_CACHE = {}


def kernel(X, wq_w, wq_b, wk_w, wk_b):
    from concourse.bass_utils import run_bass_kernel_spmd

    plan = make_plan(X)
    in_maps = make_inputs(X, wq_w, wq_b, wk_w, wk_b, plan)
    key = (tuple(plan["NB"]), tuple(plan["sideof"]), plan["TOFF"],
           tuple(plan["tlen"]), plan["NW2"])
    if key not in _CACHE:
        _CACHE[key] = build_graph(plan)
    nc = _CACHE[key]
    res = run_bass_kernel_spmd(nc, in_maps, core_ids=list(range(NCORES)),
                               trace=bool(int(os.environ.get("KTRACE", "0"))))
    outs = [res.results[c]["out"] for c in range(NCORES)]
    y = scatter_outputs(plan, outs)
    if os.environ.get("KTRACE", "0") != "0":
        kernel.last_result = res
    return y[None]  # (1, T, 2)


# revision 46
# speedup vs baseline: 1.1138x; 1.1138x over previous
"""Trainium2 Bass kernel for cumulative-state (linear) attention over M modalities.

Math (reference): out[i, e] = sum_m sum_{j : t2_m[j] <= t1[i]} (Q[i] . K_m[j]) * X_m[j, e],
for e in {0, 1}, where Q = mlp_q(X[0]), K_m = mlp_km(X[m]), t1 = X[0,:,-1], t2_m = X[m,:,-1].

Sharding: 8 cores = (m, h): modality m in 0..3, key-half h in 0..1. Each core owns
keys j in [h*4096, (h+1)*4096) of modality m and computes partial contributions for
ALL queries; the host scatter-sums the 8 partial outputs (the "all-reduce").

Per core the 4096 keys are split into NKC=33 chunks of CK=126 keys (last chunk 64).
Each chunk occupies a 128-column slot in the packed key layout: 126 key columns plus
2 reserved columns that are later overwritten with the chunk's running prefix state
srun_k (64, 2). Query i with idx in chunk k gets its full contribution from ONE
masked matmul pair:
  B = [K_chunk | srun_k]^T q          (pB: 128 rows = 126 keys + 2 state rows)
  out = [V2_chunk ; I2]^T (B * mask)  (po: mask rows 126,127 are all-ones)
so the state term and the intra-chunk causal term come out of a single accumulation.

The chunk states S_k = K_k^T V2_k are computed as 5 grouped matmuls (8 chunks per
matmul, diagonal blocks extracted), prefix-summed with ONE lower-triangular constant
matmul (no serial DVE chain), and transposed back via an identity matmul. Everything
64-contract is row-packed two-sides-per-128-partitions with concurrent quadrant
matmul pairs. Epilogues and copies are spread over ACT/DVE/GPSIMD.
"""

import os
from contextlib import ExitStack

import ml_dtypes
import numpy as np

BF16 = ml_dtypes.bfloat16

M, T, D = 4, 8192, 64
NLIN = 3
CK = 126         # keys per chunk
CH = 128         # chunk column stride (126 keys + 2 srun slots)
NK = T // 2      # keys per core (4096)
NKC = 33         # chunks per core (32*126 + 64)
NCORES = 8
FMAX = 512       # max matmul free dim / PSUM bank cols (f32)
NSMAX = 17       # max chunks per side
KW = NSMAX * CH  # packed key cols per side (2176)
NG = 4           # psc groups of 8 chunks (+1 straggler chunk 32)
NR = 2 * (NKC + 1)  # srunT rows (68)


def _round_up(x, k):
    return ((x + k - 1) // k) * k


def _scrow(k):
    """scT row for chunk k (gapped layout from tile_position col offsets)."""
    g, i = divmod(k, 8)
    return 32 * g + 2 * i


def make_plan(X):
    """Host-side: band structure + packed column layout, shared across cores."""
    X = np.asarray(X, np.float32)
    t1 = X[0, :, -1]
    los, his, tbs, idxs = [], [], [], []
    for c in range(NCORES):
        m, h = c // 2, c % 2
        t2 = X[m, :, -1]
        idx = np.searchsorted(t2, t1, side="right") - 1
        idxs.append(idx)
        hs = h * NK
        edges = [hs + min(k * CK, NK) for k in range(NKC + 1)]
        lo = np.searchsorted(idx, edges[:-1], side="left")
        hi = np.searchsorted(idx, edges[1:], side="left")
        los.append(lo)
        his.append(hi)
        tbs.append(int(np.searchsorted(idx, hs + NK, side="left")))

    NB = [0] * NKC
    for k in range(NKC):
        w = max(his[c][k] - los[c][k] for c in range(NCORES))
        NB[k] = _round_up(int(w), 8)

    # split chunks into two sides (partition halves) with balanced band totals
    order = sorted(range(NKC), key=lambda k: -NB[k])
    sideof = [0] * NKC
    tot = [0, 0]
    cnt = [0, 0]
    for k in order:
        s = 0 if (tot[0] <= tot[1] and cnt[0] < NSMAX) or cnt[1] >= NSMAX else 1
        sideof[k] = s
        tot[s] += NB[k]
        cnt[s] += 1
    TOFF = _round_up(max(tot[0], tot[1]), 8)

    kpos = [0] * NKC
    qoff = [0] * NKC
    acc = [0, 0]
    pos = [0, 0]
    for k in range(NKC):  # global ascending within each side
        s = sideof[k]
        kpos[k] = pos[s]
        qoff[k] = acc[s]
        pos[s] += 1
        acc[s] += NB[k]

    NT = _round_up(max(T - tb for tb in tbs), 8)
    tl0 = min(_round_up((NT + 1) // 2, 8), NT)
    tlen = [tl0, NT - tl0]
    NW2 = TOFF + max(tlen)
    korder = sorted(range(NKC), key=lambda k: (sideof[k], qoff[k]))

    return dict(NB=NB, TOFF=TOFF, NW2=NW2, sideof=sideof, kpos=kpos, qoff=qoff,
                tlen=tlen, los=los, his=his, tbs=tbs, idxs=idxs, korder=korder,
                lb=tot[0], rb=tot[1])


def make_inputs(X, wq_w, wq_b, wk_w, wk_b, plan):
    X = np.asarray(X, np.float32)
    wq_w = np.asarray(wq_w, np.float32)
    wq_b = np.asarray(wq_b, np.float32)
    wk_w = np.asarray(wk_w, np.float32)
    wk_b = np.asarray(wk_b, np.float32)
    NB, TOFF, NW2 = plan["NB"], plan["TOFF"], plan["NW2"]
    sideof, kpos, qoff = plan["sideof"], plan["kpos"], plan["qoff"]
    tlen = plan["tlen"]

    # weights stacked into both partition halves
    wq1 = np.concatenate([wq_w[l] for l in range(NLIN)], axis=1)
    wq = np.concatenate([wq1, wq1], axis=0).astype(BF16)              # (128, 192)
    bq1 = np.stack([wq_b[l] for l in range(NLIN)], axis=1)
    bq = np.concatenate([bq1, bq1], axis=0).astype(np.float32)        # (128, 3)

    # prefix-sum matrices, one per within-group chunk position i: row
    # 32g+2i+e of stripe i contributes chunk 8g+i's S to srunT[2k'+e] for
    # k' > 8g+i. Zero rows mask the off-diagonal garbage in the stripes.
    lmatA = np.zeros((128, 8 * NR), np.float32)
    lmatB = np.zeros((2, NR), np.float32)
    for k in range(32):
        g, i = divmod(k, 8)
        for kp in range(k + 1, NKC + 1):
            for e in range(2):
                lmatA[32 * g + 2 * i + e, i * NR + 2 * kp + e] = 1.0
    for kp in range(33, NKC + 1):
        lmatB[0, 2 * kp] = 1.0
        lmatB[1, 2 * kp + 1] = 1.0
    lmatA = lmatA.astype(BF16)
    lmatB = lmatB.astype(BF16)
    # transpose-permutation: pt col j holds srun of the j-th chunk in
    # (side, kpos) order, so the kt injection is one strided copy per side
    ns0 = sum(1 for s in sideof if s == 0)
    p68 = np.zeros((NR, NR), BF16)
    for k in range(NKC):
        j0 = (0 if sideof[k] == 0 else 2 * ns0) + 2 * kpos[k]
        p68[2 * k, j0] = 1.0
        p68[2 * k + 1, j0 + 1] = 1.0
    p68[2 * NKC, 2 * NKC] = 1.0
    p68[2 * NKC + 1, 2 * NKC + 1] = 1.0

    in_maps = []
    for c in range(NCORES):
        m, h = c // 2, c % 2
        hs = h * NK
        lo, hi, tb = plan["los"][c], plan["his"][c], plan["tbs"][c]
        idx = plan["idxs"][c]

        qb = np.zeros((2 * D, NW2), BF16)
        msk = np.zeros((CH, 2 * TOFF), BF16)
        for k in range(NKC):
            n = hi[k] - lo[k]
            s, o = sideof[k], qoff[k]
            if n > 0:
                qb[64 * s:64 * s + 64, o:o + n] = X[0, lo[k]:hi[k], :].T.astype(BF16)
                jg = hs + k * CK + np.arange(CK)[:, None]
                msk[0:CK, s * TOFF + o:s * TOFF + o + n] = \
                    (jg <= idx[None, lo[k]:hi[k]]).astype(BF16)
                msk[CK:CH, s * TOFF + o:s * TOFF + o + n] = 1.0
        # tail: first tlen[0] tail queries on side 0, rest on side 1
        ntail = T - tb
        n0 = min(ntail, tlen[0])
        if n0 > 0:
            qb[0:64, TOFF:TOFF + n0] = X[0, tb:tb + n0, :].T.astype(BF16)
        n1 = ntail - n0
        if n1 > 0:
            qb[64:128, TOFF:TOFF + n1] = X[0, tb + n0:, :].T.astype(BF16)

        xk = X[m, hs:hs + NK, :]
        xkt = np.zeros((2 * D, KW), BF16)
        v2 = np.zeros((CH, 2 * NKC), BF16)
        for k in range(NKC):
            s, p = sideof[k], kpos[k]
            a, b = k * CK, min((k + 1) * CK, NK)
            nk = b - a
            xkt[64 * s:64 * s + 64, p * CH:p * CH + nk] = xk[a:b, :].T.astype(BF16)
            v2[0:nk, 2 * k:2 * k + 2] = xk[a:b, 0:2].astype(BF16)
            v2[CK, 2 * k] = 1.0      # I2 rows: pass srun rows of bm through po
            v2[CK + 1, 2 * k + 1] = 1.0

        wk1 = np.concatenate([wk_w[m, l] for l in range(NLIN)], axis=1)
        wk = np.concatenate([wk1, wk1], axis=0).astype(BF16)          # (128, 192)
        bk1 = np.stack([wk_b[m, l] for l in range(NLIN)], axis=1)
        bk = np.concatenate([bk1, bk1], axis=0).astype(np.float32)    # (128, 3)

        # host-computed S correction: the last K-linear's bias contributes
        # b3 (x) sum_j v2[j,:] per chunk — prefix-accumulated on host and
        # added once to srunT after the triangular matmul
        b3 = wk_b[m, NLIN - 1]                                        # (64,)
        stc = np.zeros((NR, D), np.float32)
        acc = np.zeros((2, D), np.float32)
        for k in range(NKC + 1):
            stc[2 * k] = acc[0]
            stc[2 * k + 1] = acc[1]
            if k < NKC:
                a, b = k * CK, min((k + 1) * CK, NK)
                vs = xk[a:b, 0:2].astype(BF16).astype(np.float32).sum(axis=0)
                acc[0] += b3 * vs[0]
                acc[1] += b3 * vs[1]

        in_maps.append(dict(qb=qb, msk=msk, xkt=xkt, v2=v2,
                            wq=wq, bq=bq, wk=wk, bk=bk, stc=stc,
                            lmatA=lmatA, lmatB=lmatB, p68=p68))
    return in_maps


def scatter_outputs(plan, outs):
    """Host-side 'all-reduce': scatter per-core (4, NW2) partials to (T, 2)."""
    NB, TOFF = plan["NB"], plan["TOFF"]
    sideof, qoff, tlen = plan["sideof"], plan["qoff"], plan["tlen"]
    y = np.zeros((T, 2), np.float32)
    for c in range(NCORES):
        o4 = np.asarray(outs[c], np.float32)
        lo, hi, tb = plan["los"][c], plan["his"][c], plan["tbs"][c]
        for k in range(NKC):
            n = hi[k] - lo[k]
            if n > 0:
                s, o = sideof[k], qoff[k]
                y[lo[k]:hi[k], :] += o4[2 * s:2 * s + 2, o:o + n].T
        ntail = T - tb
        n0 = min(ntail, tlen[0])
        if n0 > 0:
            y[tb:tb + n0, :] += o4[0:2, TOFF:TOFF + n0].T
        n1 = ntail - n0
        if n1 > 0:
            y[tb + n0:, :] += o4[2:4, TOFF:TOFF + n1].T
    return y


# ---------------------------------------------------------------- numpy emulation
def emulate_core(im, plan):
    """Numpy mirror of the device graph for one core (f32 math, for validation)."""
    NB, TOFF, NW2 = plan["NB"], plan["TOFF"], plan["NW2"]
    sideof, kpos, qoff = plan["sideof"], plan["kpos"], plan["qoff"]
    tlen = plan["tlen"]

    def f(x):
        return np.asarray(x, np.float32)

    wk, bk, wq, bq = f(im["wk"]), f(im["bk"]), f(im["wq"]), f(im["bq"])
    qb, xkt, v2, msk = f(im["qb"]), f(im["xkt"]), f(im["v2"]), f(im["msk"])
    lmatA, lmatB = f(im["lmatA"]), f(im["lmatB"])

    def mlp_packed(xp, w, b):
        a = xp
        outs = []
        for l in range(NLIN):
            z = np.concatenate([
                w[0:64, 64 * l:64 * (l + 1)].T @ a[0:64] + b[0:64, l][:, None],
                w[64:128, 64 * l:64 * (l + 1)].T @ a[64:128] + b[64:128, l][:, None],
            ], axis=0)
            a = np.maximum(z, 0.0) if l < NLIN - 1 else z
            outs.append(a)
        return outs[-1], outs[-2]

    ktp, a2p = mlp_packed(xkt, wk, bk)       # (128, KW)
    qtp, _ = mlp_packed(qb, wq, bq)          # (128, NW2)

    # kn (keys-on-partitions, bias-free), per chunk
    kn = np.zeros((CK, D * NKC), np.float32)
    for k in range(NKC):
        s, p = sideof[k], kpos[k]
        a2 = a2p[64 * s:64 * s + 64, p * CH:p * CH + CK]
        kn[:, D * k:D * (k + 1)] = a2.T @ wk[64 * s:64 * s + 64, 2 * D:3 * D]

    # grouped psc stripes (diag blocks live at rows 32g+2i, cols 64i)
    sct1 = np.zeros((128, FMAX), np.float32)
    scTB = np.zeros((2, D), np.float32)
    for k in range(NKC):
        blk = v2[0:CK, 2 * k:2 * k + 2].T @ kn[:, D * k:D * (k + 1)]  # (2, 64)
        if k < 32:
            g, i = divmod(k, 8)
            sct1[32 * g + 2 * i:32 * g + 2 * i + 2, D * i:D * (i + 1)] = blk
        else:
            scTB = blk
    srunT = f(im["stc"]).copy()                             # (68, 64)
    for i in range(8):
        srunT += lmatA[:, i * NR:(i + 1) * NR].T @ sct1[:, D * i:D * (i + 1)]
    srunT += lmatB.T @ scTB
    srunP = srunT.T @ f(im["p68"])                          # (64, 68) side-ordered

    # inject srun into the 2 reserved kt columns of each chunk (strided)
    ns0 = sum(1 for s in sideof if s == 0)
    for k in range(NKC):
        s, p = sideof[k], kpos[k]
        j0 = (0 if s == 0 else 2 * ns0) + 2 * p
        ktp[64 * s:64 * s + 64, p * CH + CK:p * CH + CH] = srunP[:, j0:j0 + 2]
    srun_tot = srunP[:, 2 * NKC:2 * NKC + 2]

    out = np.zeros((66, NW2), np.float32)
    # tails (total-state term)
    for s in range(2):
        n = tlen[s]
        if n > 0:
            out[64 * s:64 * s + 2, TOFF:TOFF + n] = \
                srun_tot.T @ qtp[64 * s:64 * s + 64, TOFF:TOFF + n]
    # band chunks: one masked matmul pair per chunk
    for k in range(NKC):
        nq = NB[k]
        if nq == 0:
            continue
        s, p, o = sideof[k], kpos[k], qoff[k]
        qblk = qtp[64 * s:64 * s + 64, o:o + nq]
        B = ktp[64 * s:64 * s + 64, p * CH:(p + 1) * CH].T @ qblk    # (128, nq)
        bm = B * msk[:, s * TOFF + o:s * TOFF + o + nq]
        out[64 * s:64 * s + 2, o:o + nq] = v2[:, 2 * k:2 * k + 2].T @ bm
    return np.concatenate([out[0:2], out[64:66]], axis=0)  # (4, NW2)


# ---------------------------------------------------------------- device graph
def build_graph(plan):
    import concourse.bacc as bacc
    import concourse.tile as tile
    from concourse import mybir

    NB, TOFF, NW2 = plan["NB"], plan["TOFF"], plan["NW2"]
    sideof, kpos, qoff = plan["sideof"], plan["kpos"], plan["qoff"]
    tlen, korder = plan["tlen"], plan["korder"]
    f32 = mybir.dt.float32
    bf16 = mybir.dt.bfloat16
    AF = mybir.ActivationFunctionType
    OP = mybir.AluOpType

    nc = bacc.Bacc("TRN2")
    d_qb = nc.dram_tensor("qb", [2 * D, NW2], bf16, kind="ExternalInput")
    d_msk = nc.dram_tensor("msk", [CH, 2 * TOFF], bf16, kind="ExternalInput")
    d_xkt = nc.dram_tensor("xkt", [2 * D, KW], bf16, kind="ExternalInput")
    d_v2 = nc.dram_tensor("v2", [CH, 2 * NKC], bf16, kind="ExternalInput")
    d_wq = nc.dram_tensor("wq", [2 * D, D * NLIN], bf16, kind="ExternalInput")
    d_bq = nc.dram_tensor("bq", [2 * D, NLIN], f32, kind="ExternalInput")
    d_wk = nc.dram_tensor("wk", [2 * D, D * NLIN], bf16, kind="ExternalInput")
    d_bk = nc.dram_tensor("bk", [2 * D, NLIN], f32, kind="ExternalInput")
    d_stc = nc.dram_tensor("stc", [NR, D], f32, kind="ExternalInput")
    d_lA = nc.dram_tensor("lmatA", [128, 8 * NR], bf16, kind="ExternalInput")
    d_lB = nc.dram_tensor("lmatB", [2, NR], bf16, kind="ExternalInput")
    d_p68 = nc.dram_tensor("p68", [NR, NR], bf16, kind="ExternalInput")
    ns0 = sum(1 for s in sideof if s == 0)
    ns1 = NKC - ns0
    d_out = nc.dram_tensor("out", [4, NW2], f32, kind="ExternalOutput")

    with ExitStack() as ctx:
        tc = ctx.enter_context(tile.TileContext(nc))
        const = ctx.enter_context(tc.tile_pool(name="const", bufs=1))
        big = ctx.enter_context(tc.tile_pool(name="big", bufs=1))
        work = ctx.enter_context(tc.tile_pool(name="work", bufs=3))
        pmlp = ctx.enter_context(tc.tile_pool(name="pmlp", bufs=3, space="PSUM"))
        pb = ctx.enter_context(tc.tile_pool(name="pb", bufs=2, space="PSUM"))
        ps = ctx.enter_context(tc.tile_pool(name="ps", bufs=1, space="PSUM"))
        pout = ctx.enter_context(tc.tile_pool(name="pout", bufs=2, space="PSUM"))

        wq_t = const.tile([2 * D, D * NLIN], bf16, tag="wq")
        bq_t = const.tile([2 * D, NLIN], f32, tag="bq")
        wk_t = const.tile([2 * D, D * NLIN], bf16, tag="wk")
        bk_t = const.tile([2 * D, NLIN], f32, tag="bk")
        stc_t = const.tile([NR, D], f32, tag="stc")
        lA_t = const.tile([128, 8 * NR], bf16, tag="lA")
        lB_t = const.tile([2, NR], bf16, tag="lB")
        p68_t = const.tile([NR, NR], bf16, tag="p68")
        v2_t = const.tile([CH, 2 * NKC], bf16, tag="v2")

        # input DMA triggers live on SP + GPSIMD queues ONLY: a trigger stalls
        # its queue head until a hardware DMA slot frees, and ACT/DVE must not
        # be blocked behind that (they run the MLP epilogues)
        nc.sync.dma_start(wk_t[:], d_wk[:])
        xkt_t = big.tile([2 * D, KW], bf16, tag="xkt")
        nc.sync.dma_start(bk_t[:], d_bk[:])
        nc.sync.dma_start(v2_t[:], d_v2[:])
        nc.sync.dma_start(stc_t[:], d_stc[:])
        nc.sync.dma_start(lA_t[:], d_lA[:])
        nc.sync.dma_start(lB_t[:], d_lB[:])
        nc.sync.dma_start(p68_t[:], d_p68[:])
        nc.sync.dma_start(wq_t[:], d_wq[:])
        nc.sync.dma_start(bq_t[:], d_bq[:])
        qb_t = big.tile([2 * D, NW2], bf16, tag="qb")
        msk_t = big.tile([CH, 2 * TOFF], bf16, tag="msk")

        kt_t = big.tile([2 * D, KW], bf16, tag="kt")
        a2k_t = big.tile([2 * D, KW], bf16, tag="a2k")
        qt_t = big.tile([2 * D, NW2], bf16, tag="qt")
        kn_t = big.tile([CH, D * NKC], bf16, tag="kn")
        sct1_t = big.tile([128, FMAX], bf16, tag="sct1")
        scTB_t = big.tile([2, D], bf16, tag="scTB")
        srunT_t = big.tile([NR, D], bf16, tag="srunT")
        st_t = big.tile([2 * D, 2], bf16, tag="st")
        outs_t = big.tile([66, NW2], f32, tag="outs")

        # engine cycling helper: PSUM-reading copies alternate ACT/DVE
        # (GPSIMD cannot access PSUM)
        cp_state = [0]
        def cyc_copy(dst, src):
            i = cp_state[0] = (cp_state[0] + 1) % 2
            if i == 0:
                nc.scalar.copy(dst, src)
            else:
                nc.vector.tensor_copy(dst, src)

        # PE warm-up burst: dense dummy matmuls during the initial DMA window
        # flip the HAM clock gate / p-state before real work arrives
        wup_t = work.tile([CH, FMAX], bf16, tag="wup", name="wup")
        nc.vector.memset(wup_t[:], 0.0)
        nc.gpsimd.memset(sct1_t[:], 0.0)
        dma_eng = {"sync": nc.sync, "scalar": nc.scalar,
                   "gpsimd": nc.gpsimd}[os.environ.get("KDMA", "gpsimd")]
        # the 16-engine ring drains in trigger order: xkt (gates the K mlp)
        # first, then qb (Q mlp), then msk (band phase)
        for i in range(2):
            a, b = i * (KW // 2), (i + 1) * (KW // 2)
            dma_eng.dma_start(xkt_t[:, a:b], d_xkt[:, a:b])
        NQB = 4
        qsp = _round_up((NW2 + NQB - 1) // NQB, 8)
        for i in range(NQB):
            a, b = i * qsp, min((i + 1) * qsp, NW2)
            if a < b:
                dma_eng.dma_start(qb_t[:, a:b], d_qb[:, a:b])
        NMQ = 4
        msp = _round_up((2 * TOFF + NMQ - 1) // NMQ, 8)
        for i in range(NMQ):
            a, b = i * msp, min((i + 1) * msp, 2 * TOFF)
            if a < b:
                dma_eng.dma_start(msk_t[:, a:b], d_msk[:, a:b])
        for _ in range(int(os.environ.get("KWUP", "4"))):
            pwu = pb.tile([CH, FMAX], f32, tag="pb", name="pwu")
            nc.tensor.matmul(pwu[:], wup_t[:, 0:CH], wup_t[:], start=True, stop=True)

        ep_state = [0]

        def emit_layer(l, src_t, dst_t, w_t, b_t, n_cols):
            """One row-packed MLP layer: stream all column blocks through the
            PE back-to-back (same stationary weights), bias+relu epilogues
            alternating ACT/DVE (ACT is faster: give it 2 of 3)."""
            for a in range(0, n_cols, FMAX):
                b = min(a + FMAX, n_cols)
                n = b - a
                cur = src_t[:, a:b]
                pz = pmlp.tile([CH, FMAX], f32, tag="pmlp", name="pz")
                nc.tensor.matmul(pz[0:64, :n], w_t[0:64, D * l:D * (l + 1)],
                                 cur[0:64, :], start=True, stop=True,
                                 tile_position=(0, 0))
                nc.tensor.matmul(pz[64:128, :n], w_t[64:128, D * l:D * (l + 1)],
                                 cur[64:128, :], start=True, stop=True,
                                 tile_position=(64, 64))
                dst = dst_t[:, a:b]
                e = ep_state[0] = (ep_state[0] + 1) % 3
                if e == 1:
                    if l < NLIN - 1:
                        nc.vector.tensor_scalar(dst, pz[:, :n],
                                                b_t[:, l:l + 1], 0.0,
                                                OP.add, OP.max)
                    else:
                        nc.vector.tensor_scalar_add(dst, pz[:, :n],
                                                    b_t[:, l:l + 1])
                else:
                    nc.scalar.activation(dst, pz[:, :n],
                                         AF.Relu if l < NLIN - 1 else AF.Identity,
                                         bias=b_t[:, l:l + 1])

        # K mlp
        mlpk_t = work.tile([CH, KW], bf16, tag="mlpa", name="mlpk")
        emit_layer(0, xkt_t, mlpk_t, wk_t, bk_t, KW)
        emit_layer(1, mlpk_t, a2k_t, wk_t, bk_t, KW)
        emit_layer(2, a2k_t, kt_t, wk_t, bk_t, KW)

        # kn: K natural per chunk (bias-free; bias folded into stc), two
        # chunks per PSUM tile so the drain copies halve
        if os.environ.get("KKNP", "0") != "0":
            for k0 in range(0, NKC, 2):
                pkn = pb.tile([CH, FMAX], f32, tag="pb", name="pkn")
                for k in (k0, k0 + 1):
                    if k >= NKC:
                        continue
                    s, p = sideof[k], kpos[k]
                    nc.tensor.matmul(pkn[0:CK, D * (k - k0):D * (k - k0 + 1)],
                                     a2k_t[64 * s:64 * s + 64, CH * p:CH * p + CK],
                                     wk_t[64 * s:64 * s + 64, 2 * D:3 * D],
                                     start=True, stop=True,
                                     tile_position=(64 * s, 0))
                n = D * min(2, NKC - k0)
                cyc_copy(kn_t[0:CK, D * k0:D * k0 + n], pkn[0:CK, :n])
        else:
            for k in range(NKC):
                s, p = sideof[k], kpos[k]
                pkn = pb.tile([CH, FMAX], f32, tag="pb", name="pkn")
                nc.tensor.matmul(pkn[0:CK, :D],
                                 a2k_t[64 * s:64 * s + 64, CH * p:CH * p + CK],
                                 wk_t[64 * s:64 * s + 64, 2 * D:3 * D],
                                 start=True, stop=True, tile_position=(64 * s, 0))
                cyc_copy(kn_t[0:CK, D * k:D * (k + 1)], pkn[0:CK, :D])

        # grouped S chunks: 4 matmuls of 8 chunks (diag blocks) + straggler
        psc = ps.tile([128, FMAX], f32, tag="ps", name="psc")
        for g in range(NG):
            nc.tensor.matmul(psc[32 * g:32 * g + 16, 0:FMAX],
                             v2_t[0:CK, 16 * g:16 * g + 16],
                             kn_t[0:CK, FMAX * g:FMAX * (g + 1)],
                             start=True, stop=True, tile_position=(0, 32 * g))
        psB = pb.tile([CH, FMAX], f32, tag="pb", name="psB")
        nc.tensor.matmul(psB[0:2, :D], v2_t[0:CK, 64:66],
                         kn_t[0:CK, D * 32:D * 33],
                         start=True, stop=True, tile_position=(0, 0))
        # stripe drain: PSUM group stripes -> SBUF bf16 (32-aligned partition
        # starts; finer PSUM reads are illegal). Gap rows were memset once so
        # the prefix matmuls' zero weights never hit NaN garbage.
        for g in range(NG):
            cyc_copy(sct1_t[32 * g:32 * g + 16, :], psc[32 * g:32 * g + 16, :])
        cyc_copy(scTB_t[:], psB[0:2, :D])

        # Q mlp layer 1 keeps the PE busy across the srun dependency chain
        mlpq_t = work.tile([CH, NW2], bf16, tag="mlpa", name="mlpq")
        emit_layer(0, qb_t, mlpq_t, wq_t, bq_t, NW2)

        # prefix sums via 9 accumulating triangular matmuls straight off the
        # stripes (lmatA_i's zero rows mask the off-diagonal stripe garbage),
        # then the host bias-prefix correction is added in the PSUM drain
        pl = pb.tile([CH, FMAX], f32, tag="pb", name="pl")
        for i in range(8):
            nc.tensor.matmul(pl[0:NR, :D], lA_t[:, NR * i:NR * (i + 1)],
                             sct1_t[:, D * i:D * (i + 1)],
                             start=(i == 0), stop=False, tile_position=(0, 0))
        nc.tensor.matmul(pl[0:NR, :D], lB_t[:], scTB_t[:], start=False, stop=True,
                         tile_position=(0, 0))
        nc.vector.tensor_add(srunT_t[:], pl[0:NR, :D], stc_t[:])

        # transpose back via the side-order permutation matmul, duplicated
        # into both halves; srun lands pre-sorted for one strided copy/side
        pt = pb.tile([CH, FMAX], f32, tag="pb", name="pt")
        nc.tensor.matmul(pt[0:64, 0:NR], srunT_t[:], p68_t[:],
                         start=True, stop=True, tile_position=(0, 0))
        nc.tensor.matmul(pt[64:128, 0:NR], srunT_t[:], p68_t[:],
                         start=True, stop=True, tile_position=(0, 64))
        # inject srun_k into every chunk's 2 reserved kt columns: one strided
        # copy per side (33 tiny copies serialized horribly here before)
        if os.environ.get("KINJ", "strided") == "strided":
            for s, nss, base in ((0, ns0, 0), (1, ns1, 2 * ns0)):
                dst = kt_t[64 * s:64 * s + 64, 0:KW].rearrange(
                    "p (c w) -> p c w", w=CH)[:, 0:nss, CK:CH]
                src = pt[64 * s:64 * s + 64, base:base + 2 * nss].rearrange(
                    "p (c two) -> p c two", two=2)
                if s == 0:
                    nc.scalar.copy(dst, src)
                else:
                    nc.vector.tensor_copy(dst, src)
        else:
            for k in range(NKC):
                s, p = sideof[k], kpos[k]
                j0 = (0 if s == 0 else 2 * ns0) + 2 * p
                cyc_copy(kt_t[64 * s:64 * s + 64, CH * p + CK:CH * p + CH],
                         pt[64 * s:64 * s + 64, j0:j0 + 2])
        # total state for the tails
        nc.scalar.copy(st_t[0:64, :], pt[0:64, 2 * NKC:2 * NKC + 2])
        nc.scalar.copy(st_t[64:128, :], pt[64:128, 2 * NKC:2 * NKC + 2])

        # Q mlp layers 2-3 (mid shares qt_t — Tile serializes per-block WAR)
        emit_layer(1, mlpq_t, qt_t, wq_t, bq_t, NW2)
        emit_layer(2, qt_t, qt_t, wq_t, bq_t, NW2)

        # tails (state-only), sides paired on PE quadrants
        for a in range(0, max(tlen), FMAX):
            for s in range(2):
                b = min(a + FMAX, tlen[s])
                if a >= b:
                    continue
                n = b - a
                po = pout.tile([CH, FMAX], f32, tag="pout", name="po_t")
                nc.tensor.matmul(po[64 * s:64 * s + 2, :n], st_t[64 * s:64 * s + 64, :],
                                 qt_t[64 * s:64 * s + 64, TOFF + a:TOFF + b],
                                 start=True, stop=True,
                                 tile_position=(64 * s, 64 * s))
                cyc_copy(outs_t[64 * s:64 * s + 2, TOFF + a:TOFF + b],
                         po[64 * s:64 * s + 2, :n])
        for s in range(2):
            if tlen[s] > 0:
                nc.sync.dma_start(d_out[2 * s:2 * s + 2, TOFF:TOFF + tlen[s]],
                                  outs_t[64 * s:64 * s + 2, TOFF:TOFF + tlen[s]])

        # band chunks: one masked matmul pair per chunk, software-pipelined
        # one deep so the PE never stalls on the DVE mask multiply
        segs = []
        for k in korder:
            nq = NB[k]
            s, p, o0 = sideof[k], kpos[k], int(qoff[k])
            for a in range(0, nq, FMAX):
                segs.append((k, s, p, o0 + a, min(a + FMAX, nq) - a))
        pBs = [None] * len(segs)

        def emit_pB(i):
            k, s, p, o, n = segs[i]
            pB = pBs[i] = pb.tile([CH, FMAX], f32, tag="pb", name="pB")
            nc.tensor.matmul(pB[:, :n], kt_t[64 * s:64 * s + 64, CH * p:CH * (p + 1)],
                             qt_t[64 * s:64 * s + 64, o:o + n],
                             start=True, stop=True, tile_position=(64 * s, 0))

        def emit_rest(i):
            k, s, p, o, n = segs[i]
            bm = work.tile([CH, FMAX], bf16, tag="bm", name="bm")
            nc.vector.tensor_mul(bm[:, :n], pBs[i][:, :n],
                                 msk_t[:, s * TOFF + o:s * TOFF + o + n])
            po = pout.tile([CH, FMAX], f32, tag="pout", name="po")
            nc.tensor.matmul(po[64 * s:64 * s + 2, :n], v2_t[:, 2 * k:2 * k + 2],
                             bm[:, :n], start=True, stop=True,
                             tile_position=(0, 64 * s))
            cyc_copy(outs_t[64 * s:64 * s + 2, o:o + n],
                     po[64 * s:64 * s + 2, :n])

        if os.environ.get("KSEQ", "0") != "0":
            for i in range(len(segs)):
                emit_pB(i)
                emit_rest(i)
        elif segs:
            emit_pB(0)
            for i in range(1, len(segs)):
                emit_pB(i)
                emit_rest(i - 1)
            emit_rest(len(segs) - 1)
        for s in range(2):
            w = plan["lb"] if s == 0 else plan["rb"]
            if w > 0:
                nc.sync.dma_start(d_out[2 * s:2 * s + 2, 0:w],
                                  outs_t[64 * s:64 * s + 2, 0:w])

    nc.finalize()
    return nc


_CACHE = {}


def kernel(X, wq_w, wq_b, wk_w, wk_b):
    from concourse.bass_utils import run_bass_kernel_spmd

    plan = make_plan(X)
    in_maps = make_inputs(X, wq_w, wq_b, wk_w, wk_b, plan)
    key = (tuple(plan["NB"]), tuple(plan["sideof"]), plan["TOFF"],
           tuple(plan["tlen"]), plan["NW2"])
    if key not in _CACHE:
        _CACHE[key] = build_graph(plan)
    nc = _CACHE[key]
    res = run_bass_kernel_spmd(nc, in_maps, core_ids=list(range(NCORES)),
                               trace=bool(int(os.environ.get("KTRACE", "0"))))
    outs = [res.results[c]["out"] for c in range(NCORES)]
    y = scatter_outputs(plan, outs)
    if os.environ.get("KTRACE", "0") != "0":
        kernel.last_result = res
    return y[None]  # (1, T, 2)


# revision 48
# speedup vs baseline: 1.1578x; 1.0395x over previous
"""Trainium2 Bass kernel for cumulative-state (linear) attention over M modalities.

Math (reference): out[i, e] = sum_m sum_{j : t2_m[j] <= t1[i]} (Q[i] . K_m[j]) * X_m[j, e],
for e in {0, 1}, where Q = mlp_q(X[0]), K_m = mlp_km(X[m]), t1 = X[0,:,-1], t2_m = X[m,:,-1].

Sharding: 8 cores = (m, h): modality m in 0..3, key-half h in 0..1. Each core owns
keys j in [h*4096, (h+1)*4096) of modality m and computes partial contributions for
ALL queries; the host scatter-sums the 8 partial outputs (the "all-reduce").

Per core the 4096 keys are split into NKC=33 chunks of CK=126 keys (last chunk 64).
Each chunk occupies a 128-column slot in the packed key layout: 126 key columns plus
2 reserved columns that are later overwritten with the chunk's running prefix state
srun_k (64, 2). Query i with idx in chunk k gets its full contribution from ONE
masked matmul pair:
  B = [K_chunk | srun_k]^T q          (pB: 128 rows = 126 keys + 2 state rows)
  out = [V2_chunk ; I2]^T (B * mask)  (po: mask rows 126,127 are all-ones)
so the state term and the intra-chunk causal term come out of a single accumulation.

The chunk states S_k = K_k^T V2_k are computed as 5 grouped matmuls (8 chunks per
matmul, diagonal blocks extracted), prefix-summed with ONE lower-triangular constant
matmul (no serial DVE chain), and transposed back via an identity matmul. Everything
64-contract is row-packed two-sides-per-128-partitions with concurrent quadrant
matmul pairs. Epilogues and copies are spread over ACT/DVE/GPSIMD.
"""

import os
from contextlib import ExitStack

import ml_dtypes
import numpy as np

BF16 = ml_dtypes.bfloat16

M, T, D = 4, 8192, 64
NLIN = 3
CK = 126         # keys per chunk
CH = 128         # chunk column stride (126 keys + 2 srun slots)
NK = T // 2      # keys per core (4096)
NKC = 33         # chunks per core (32*126 + 64)
NCORES = 8
FMAX = 512       # max matmul free dim / PSUM bank cols (f32)
NSMAX = 17       # max chunks per side
KW = NSMAX * CH  # packed key cols per side (2176)
NG = 4           # psc groups of 8 chunks (+1 straggler chunk 32)
NR = 2 * (NKC + 1)  # srunT rows (68)


def _round_up(x, k):
    return ((x + k - 1) // k) * k


def _scrow(k):
    """scT row for chunk k (gapped layout from tile_position col offsets)."""
    g, i = divmod(k, 8)
    return 32 * g + 2 * i


def make_plan(X):
    """Host-side: band structure + packed column layout, shared across cores."""
    X = np.asarray(X, np.float32)
    t1 = X[0, :, -1]
    los, his, tbs, idxs = [], [], [], []
    for c in range(NCORES):
        m, h = c // 2, c % 2
        t2 = X[m, :, -1]
        idx = np.searchsorted(t2, t1, side="right") - 1
        idxs.append(idx)
        hs = h * NK
        edges = [hs + min(k * CK, NK) for k in range(NKC + 1)]
        lo = np.searchsorted(idx, edges[:-1], side="left")
        hi = np.searchsorted(idx, edges[1:], side="left")
        los.append(lo)
        his.append(hi)
        tbs.append(int(np.searchsorted(idx, hs + NK, side="left")))

    NB = [0] * NKC
    for k in range(NKC):
        w = max(his[c][k] - los[c][k] for c in range(NCORES))
        NB[k] = _round_up(int(w), 8)

    # split chunks into two sides (partition halves) with balanced band totals
    order = sorted(range(NKC), key=lambda k: -NB[k])
    sideof = [0] * NKC
    tot = [0, 0]
    cnt = [0, 0]
    for k in order:
        s = 0 if (tot[0] <= tot[1] and cnt[0] < NSMAX) or cnt[1] >= NSMAX else 1
        sideof[k] = s
        tot[s] += NB[k]
        cnt[s] += 1
    TOFF = _round_up(max(tot[0], tot[1]), 8)

    kpos = [0] * NKC
    qoff = [0] * NKC
    acc = [0, 0]
    pos = [0, 0]
    for k in range(NKC):  # global ascending within each side
        s = sideof[k]
        kpos[k] = pos[s]
        qoff[k] = acc[s]
        pos[s] += 1
        acc[s] += NB[k]

    NT = _round_up(max(T - tb for tb in tbs), 8)
    tl0 = min(_round_up((NT + 1) // 2, 8), NT)
    tlen = [tl0, NT - tl0]
    NW2 = TOFF + max(tlen)
    korder = sorted(range(NKC), key=lambda k: (sideof[k], qoff[k]))

    return dict(NB=NB, TOFF=TOFF, NW2=NW2, sideof=sideof, kpos=kpos, qoff=qoff,
                tlen=tlen, los=los, his=his, tbs=tbs, idxs=idxs, korder=korder,
                lb=tot[0], rb=tot[1])


def make_inputs(X, wq_w, wq_b, wk_w, wk_b, plan):
    X = np.asarray(X, np.float32)
    wq_w = np.asarray(wq_w, np.float32)
    wq_b = np.asarray(wq_b, np.float32)
    wk_w = np.asarray(wk_w, np.float32)
    wk_b = np.asarray(wk_b, np.float32)
    NB, TOFF, NW2 = plan["NB"], plan["TOFF"], plan["NW2"]
    sideof, kpos, qoff = plan["sideof"], plan["kpos"], plan["qoff"]
    tlen = plan["tlen"]

    # weights stacked into both partition halves
    wq1 = np.concatenate([wq_w[l] for l in range(NLIN)], axis=1)
    wq = np.concatenate([wq1, wq1], axis=0).astype(BF16)              # (128, 192)
    bq1 = np.stack([wq_b[l] for l in range(NLIN)], axis=1)
    bq = np.concatenate([bq1, bq1], axis=0).astype(np.float32)        # (128, 3)

    # prefix-sum matrices, one per within-group chunk position i: row
    # 32g+2i+e of stripe i contributes chunk 8g+i's S to srunT[2k'+e] for
    # k' > 8g+i. Zero rows mask the off-diagonal garbage in the stripes.
    lmatA = np.zeros((128, 8 * NR), np.float32)
    lmatB = np.zeros((2, NR), np.float32)
    for k in range(32):
        g, i = divmod(k, 8)
        for kp in range(k + 1, NKC + 1):
            for e in range(2):
                lmatA[32 * g + 2 * i + e, i * NR + 2 * kp + e] = 1.0
    for kp in range(33, NKC + 1):
        lmatB[0, 2 * kp] = 1.0
        lmatB[1, 2 * kp + 1] = 1.0
    lmatA = lmatA.astype(BF16)
    lmatB = lmatB.astype(BF16)
    # transpose-permutation: pt col j holds srun of the j-th chunk in
    # (side, kpos) order, so the kt injection is one strided copy per side
    ns0 = sum(1 for s in sideof if s == 0)
    p68 = np.zeros((NR, NR), BF16)
    for k in range(NKC):
        j0 = (0 if sideof[k] == 0 else 2 * ns0) + 2 * kpos[k]
        p68[2 * k, j0] = 1.0
        p68[2 * k + 1, j0 + 1] = 1.0
    p68[2 * NKC, 2 * NKC] = 1.0
    p68[2 * NKC + 1, 2 * NKC + 1] = 1.0

    in_maps = []
    for c in range(NCORES):
        m, h = c // 2, c % 2
        hs = h * NK
        lo, hi, tb = plan["los"][c], plan["his"][c], plan["tbs"][c]
        idx = plan["idxs"][c]

        qb = np.zeros((2 * D, NW2), BF16)
        msk = np.zeros((CH, 2 * TOFF), BF16)
        for k in range(NKC):
            n = hi[k] - lo[k]
            s, o = sideof[k], qoff[k]
            if n > 0:
                qb[64 * s:64 * s + 64, o:o + n] = X[0, lo[k]:hi[k], :].T.astype(BF16)
                jg = hs + k * CK + np.arange(CK)[:, None]
                msk[0:CK, s * TOFF + o:s * TOFF + o + n] = \
                    (jg <= idx[None, lo[k]:hi[k]]).astype(BF16)
                msk[CK:CH, s * TOFF + o:s * TOFF + o + n] = 1.0
        # tail: first tlen[0] tail queries on side 0, rest on side 1
        ntail = T - tb
        n0 = min(ntail, tlen[0])
        if n0 > 0:
            qb[0:64, TOFF:TOFF + n0] = X[0, tb:tb + n0, :].T.astype(BF16)
        n1 = ntail - n0
        if n1 > 0:
            qb[64:128, TOFF:TOFF + n1] = X[0, tb + n0:, :].T.astype(BF16)

        xk = X[m, hs:hs + NK, :]
        xkt = np.zeros((2 * D, KW), BF16)
        v2 = np.zeros((CH, 2 * NKC), BF16)
        for k in range(NKC):
            s, p = sideof[k], kpos[k]
            a, b = k * CK, min((k + 1) * CK, NK)
            nk = b - a
            xkt[64 * s:64 * s + 64, p * CH:p * CH + nk] = xk[a:b, :].T.astype(BF16)
            v2[0:nk, 2 * k:2 * k + 2] = xk[a:b, 0:2].astype(BF16)
            v2[CK, 2 * k] = 1.0      # I2 rows: pass srun rows of bm through po
            v2[CK + 1, 2 * k + 1] = 1.0

        wk1 = np.concatenate([wk_w[m, l] for l in range(NLIN)], axis=1)
        wk = np.concatenate([wk1, wk1], axis=0).astype(BF16)          # (128, 192)
        bk1 = np.stack([wk_b[m, l] for l in range(NLIN)], axis=1)
        bk = np.concatenate([bk1, bk1], axis=0).astype(np.float32)    # (128, 3)

        # host-computed S correction: the last K-linear's bias contributes
        # b3 (x) sum_j v2[j,:] per chunk — prefix-accumulated on host and
        # added once to srunT after the triangular matmul
        b3 = wk_b[m, NLIN - 1]                                        # (64,)
        stc = np.zeros((NR, D), np.float32)
        acc = np.zeros((2, D), np.float32)
        for k in range(NKC + 1):
            stc[2 * k] = acc[0]
            stc[2 * k + 1] = acc[1]
            if k < NKC:
                a, b = k * CK, min((k + 1) * CK, NK)
                vs = xk[a:b, 0:2].astype(BF16).astype(np.float32).sum(axis=0)
                acc[0] += b3 * vs[0]
                acc[1] += b3 * vs[1]

        in_maps.append(dict(qb=qb, msk=msk, xkt=xkt, v2=v2,
                            wq=wq, bq=bq, wk=wk, bk=bk, stc=stc,
                            lmatA=lmatA, lmatB=lmatB, p68=p68))
    return in_maps


def scatter_outputs(plan, outs):
    """Host-side 'all-reduce': scatter per-core (4, NW2) partials to (T, 2)."""
    NB, TOFF = plan["NB"], plan["TOFF"]
    sideof, qoff, tlen = plan["sideof"], plan["qoff"], plan["tlen"]
    y = np.zeros((T, 2), np.float32)
    for c in range(NCORES):
        o4 = np.asarray(outs[c], np.float32)
        lo, hi, tb = plan["los"][c], plan["his"][c], plan["tbs"][c]
        for k in range(NKC):
            n = hi[k] - lo[k]
            if n > 0:
                s, o = sideof[k], qoff[k]
                y[lo[k]:hi[k], :] += o4[2 * s:2 * s + 2, o:o + n].T
        ntail = T - tb
        n0 = min(ntail, tlen[0])
        if n0 > 0:
            y[tb:tb + n0, :] += o4[0:2, TOFF:TOFF + n0].T
        n1 = ntail - n0
        if n1 > 0:
            y[tb + n0:, :] += o4[2:4, TOFF:TOFF + n1].T
    return y


# ---------------------------------------------------------------- numpy emulation
def emulate_core(im, plan):
    """Numpy mirror of the device graph for one core (f32 math, for validation)."""
    NB, TOFF, NW2 = plan["NB"], plan["TOFF"], plan["NW2"]
    sideof, kpos, qoff = plan["sideof"], plan["kpos"], plan["qoff"]
    tlen = plan["tlen"]

    def f(x):
        return np.asarray(x, np.float32)

    wk, bk, wq, bq = f(im["wk"]), f(im["bk"]), f(im["wq"]), f(im["bq"])
    qb, xkt, v2, msk = f(im["qb"]), f(im["xkt"]), f(im["v2"]), f(im["msk"])
    lmatA, lmatB = f(im["lmatA"]), f(im["lmatB"])

    def mlp_packed(xp, w, b):
        a = xp
        outs = []
        for l in range(NLIN):
            z = np.concatenate([
                w[0:64, 64 * l:64 * (l + 1)].T @ a[0:64] + b[0:64, l][:, None],
                w[64:128, 64 * l:64 * (l + 1)].T @ a[64:128] + b[64:128, l][:, None],
            ], axis=0)
            a = np.maximum(z, 0.0) if l < NLIN - 1 else z
            outs.append(a)
        return outs[-1], outs[-2]

    ktp, a2p = mlp_packed(xkt, wk, bk)       # (128, KW)
    qtp, _ = mlp_packed(qb, wq, bq)          # (128, NW2)

    # kn (keys-on-partitions, bias-free), per chunk
    kn = np.zeros((CK, D * NKC), np.float32)
    for k in range(NKC):
        s, p = sideof[k], kpos[k]
        a2 = a2p[64 * s:64 * s + 64, p * CH:p * CH + CK]
        kn[:, D * k:D * (k + 1)] = a2.T @ wk[64 * s:64 * s + 64, 2 * D:3 * D]

    # grouped psc stripes (diag blocks live at rows 32g+2i, cols 64i)
    sct1 = np.zeros((128, FMAX), np.float32)
    scTB = np.zeros((2, D), np.float32)
    for k in range(NKC):
        blk = v2[0:CK, 2 * k:2 * k + 2].T @ kn[:, D * k:D * (k + 1)]  # (2, 64)
        if k < 32:
            g, i = divmod(k, 8)
            sct1[32 * g + 2 * i:32 * g + 2 * i + 2, D * i:D * (i + 1)] = blk
        else:
            scTB = blk
    srunT = f(im["stc"]).copy()                             # (68, 64)
    for i in range(8):
        srunT += lmatA[:, i * NR:(i + 1) * NR].T @ sct1[:, D * i:D * (i + 1)]
    srunT += lmatB.T @ scTB
    srunP = srunT.T @ f(im["p68"])                          # (64, 68) side-ordered

    # inject srun into the 2 reserved kt columns of each chunk (strided)
    ns0 = sum(1 for s in sideof if s == 0)
    for k in range(NKC):
        s, p = sideof[k], kpos[k]
        j0 = (0 if s == 0 else 2 * ns0) + 2 * p
        ktp[64 * s:64 * s + 64, p * CH + CK:p * CH + CH] = srunP[:, j0:j0 + 2]
    srun_tot = srunP[:, 2 * NKC:2 * NKC + 2]

    out = np.zeros((66, NW2), np.float32)
    # tails (total-state term)
    for s in range(2):
        n = tlen[s]
        if n > 0:
            out[64 * s:64 * s + 2, TOFF:TOFF + n] = \
                srun_tot.T @ qtp[64 * s:64 * s + 64, TOFF:TOFF + n]
    # band chunks: one masked matmul pair per chunk
    for k in range(NKC):
        nq = NB[k]
        if nq == 0:
            continue
        s, p, o = sideof[k], kpos[k], qoff[k]
        qblk = qtp[64 * s:64 * s + 64, o:o + nq]
        B = ktp[64 * s:64 * s + 64, p * CH:(p + 1) * CH].T @ qblk    # (128, nq)
        bm = B * msk[:, s * TOFF + o:s * TOFF + o + nq]
        out[64 * s:64 * s + 2, o:o + nq] = v2[:, 2 * k:2 * k + 2].T @ bm
    return np.concatenate([out[0:2], out[64:66]], axis=0)  # (4, NW2)


# ---------------------------------------------------------------- device graph
def build_graph(plan):
    import concourse.bacc as bacc
    import concourse.tile as tile
    from concourse import mybir

    NB, TOFF, NW2 = plan["NB"], plan["TOFF"], plan["NW2"]
    sideof, kpos, qoff = plan["sideof"], plan["kpos"], plan["qoff"]
    tlen, korder = plan["tlen"], plan["korder"]
    f32 = mybir.dt.float32
    bf16 = mybir.dt.bfloat16
    AF = mybir.ActivationFunctionType
    OP = mybir.AluOpType

    nc = bacc.Bacc("TRN2")
    d_qb = nc.dram_tensor("qb", [2 * D, NW2], bf16, kind="ExternalInput")
    d_msk = nc.dram_tensor("msk", [CH, 2 * TOFF], bf16, kind="ExternalInput")
    d_xkt = nc.dram_tensor("xkt", [2 * D, KW], bf16, kind="ExternalInput")
    d_v2 = nc.dram_tensor("v2", [CH, 2 * NKC], bf16, kind="ExternalInput")
    d_wq = nc.dram_tensor("wq", [2 * D, D * NLIN], bf16, kind="ExternalInput")
    d_bq = nc.dram_tensor("bq", [2 * D, NLIN], f32, kind="ExternalInput")
    d_wk = nc.dram_tensor("wk", [2 * D, D * NLIN], bf16, kind="ExternalInput")
    d_bk = nc.dram_tensor("bk", [2 * D, NLIN], f32, kind="ExternalInput")
    d_stc = nc.dram_tensor("stc", [NR, D], f32, kind="ExternalInput")
    d_lA = nc.dram_tensor("lmatA", [128, 8 * NR], bf16, kind="ExternalInput")
    d_lB = nc.dram_tensor("lmatB", [2, NR], bf16, kind="ExternalInput")
    d_p68 = nc.dram_tensor("p68", [NR, NR], bf16, kind="ExternalInput")
    ns0 = sum(1 for s in sideof if s == 0)
    ns1 = NKC - ns0
    d_out = nc.dram_tensor("out", [4, NW2], f32, kind="ExternalOutput")

    with ExitStack() as ctx:
        tc = ctx.enter_context(tile.TileContext(nc))
        const = ctx.enter_context(tc.tile_pool(name="const", bufs=1))
        big = ctx.enter_context(tc.tile_pool(name="big", bufs=1))
        work = ctx.enter_context(tc.tile_pool(name="work", bufs=3))
        pmlp = ctx.enter_context(tc.tile_pool(name="pmlp", bufs=3, space="PSUM"))
        pb = ctx.enter_context(tc.tile_pool(name="pb", bufs=2, space="PSUM"))
        ps = ctx.enter_context(tc.tile_pool(name="ps", bufs=1, space="PSUM"))
        pout = ctx.enter_context(tc.tile_pool(name="pout", bufs=2, space="PSUM"))

        wq_t = const.tile([2 * D, D * NLIN], bf16, tag="wq")
        bq_t = const.tile([2 * D, NLIN], f32, tag="bq")
        wk_t = const.tile([2 * D, D * NLIN], bf16, tag="wk")
        bk_t = const.tile([2 * D, NLIN], f32, tag="bk")
        stc_t = const.tile([NR, D], f32, tag="stc")
        lA_t = const.tile([128, 8 * NR], bf16, tag="lA")
        lB_t = const.tile([2, NR], bf16, tag="lB")
        p68_t = const.tile([NR, NR], bf16, tag="p68")
        v2_t = const.tile([CH, 2 * NKC], bf16, tag="v2")

        # input DMA triggers live on SP + GPSIMD queues ONLY: a trigger stalls
        # its queue head until a hardware DMA slot frees, and ACT/DVE must not
        # be blocked behind that (they run the MLP epilogues)
        nc.sync.dma_start(wk_t[:], d_wk[:])
        xkt_t = big.tile([2 * D, KW], bf16, tag="xkt")
        nc.sync.dma_start(bk_t[:], d_bk[:])
        nc.sync.dma_start(v2_t[:], d_v2[:])
        nc.sync.dma_start(stc_t[:], d_stc[:])
        nc.sync.dma_start(lA_t[:], d_lA[:])
        nc.sync.dma_start(lB_t[:], d_lB[:])
        nc.sync.dma_start(p68_t[:], d_p68[:])
        nc.sync.dma_start(wq_t[:], d_wq[:])
        nc.sync.dma_start(bq_t[:], d_bq[:])
        qb_t = big.tile([2 * D, NW2], bf16, tag="qb")
        msk_t = big.tile([CH, 2 * TOFF], bf16, tag="msk")

        kt_t = big.tile([2 * D, KW], bf16, tag="kt")
        a2k_t = big.tile([2 * D, KW], bf16, tag="a2k")
        qt_t = big.tile([2 * D, NW2], bf16, tag="qt")
        kn_t = big.tile([CH, D * NKC], bf16, tag="kn")
        sct1_t = big.tile([128, FMAX], bf16, tag="sct1")
        scTB_t = big.tile([2, D], bf16, tag="scTB")
        srunT_t = big.tile([NR, D], bf16, tag="srunT")
        st_t = big.tile([2 * D, 2], bf16, tag="st")
        outs_t = big.tile([66, NW2], f32, tag="outs")

        # engine cycling helper: PSUM-reading copies alternate ACT/DVE
        # (GPSIMD cannot access PSUM)
        cp_state = [0]
        def cyc_copy(dst, src):
            i = cp_state[0] = (cp_state[0] + 1) % 2
            if i == 0:
                nc.scalar.copy(dst, src)
            else:
                nc.vector.tensor_copy(dst, src)

        # PE warm-up burst: dense dummy matmuls during the initial DMA window
        # flip the HAM clock gate / p-state before real work arrives
        wup_t = work.tile([CH, FMAX], bf16, tag="wup", name="wup")
        nc.vector.memset(wup_t[:], 0.0)
        nc.gpsimd.memset(sct1_t[:], 0.0)
        dma_eng = {"sync": nc.sync, "scalar": nc.scalar,
                   "gpsimd": nc.gpsimd}[os.environ.get("KDMA", "gpsimd")]
        # the 16-engine ring drains in trigger order: xkt (gates the K mlp)
        # first, then qb (Q mlp), then msk (band phase)
        for i in range(4):
            a, b = i * (KW // 4), (i + 1) * (KW // 4)
            dma_eng.dma_start(xkt_t[:, a:b], d_xkt[:, a:b])
        NQB = 4
        qsp = _round_up((NW2 + NQB - 1) // NQB, 8)
        for i in range(NQB):
            a, b = i * qsp, min((i + 1) * qsp, NW2)
            if a < b:
                dma_eng.dma_start(qb_t[:, a:b], d_qb[:, a:b])
        NMQ = 4
        msp = _round_up((2 * TOFF + NMQ - 1) // NMQ, 8)
        for i in range(NMQ):
            a, b = i * msp, min((i + 1) * msp, 2 * TOFF)
            if a < b:
                dma_eng.dma_start(msk_t[:, a:b], d_msk[:, a:b])
        for _ in range(int(os.environ.get("KWUP", "4"))):
            pwu = pb.tile([CH, FMAX], f32, tag="pb", name="pwu")
            nc.tensor.matmul(pwu[:], wup_t[:, 0:CH], wup_t[:], start=True, stop=True)

        ep_state = [0]

        def emit_layer(l, src_t, dst_t, w_t, b_t, n_cols):
            """One row-packed MLP layer: stream all column blocks through the
            PE back-to-back (same stationary weights), bias+relu epilogues
            alternating ACT/DVE (ACT is faster: give it 2 of 3)."""
            for a in range(0, n_cols, FMAX):
                b = min(a + FMAX, n_cols)
                n = b - a
                cur = src_t[:, a:b]
                pz = pmlp.tile([CH, FMAX], f32, tag="pmlp", name="pz")
                nc.tensor.matmul(pz[0:64, :n], w_t[0:64, D * l:D * (l + 1)],
                                 cur[0:64, :], start=True, stop=True,
                                 tile_position=(0, 0))
                nc.tensor.matmul(pz[64:128, :n], w_t[64:128, D * l:D * (l + 1)],
                                 cur[64:128, :], start=True, stop=True,
                                 tile_position=(64, 64))
                if os.environ.get("KFILL", "0") != "0":
                    pfil = pb.tile([CH, FMAX], f32, tag="pb", name="pfil")
                    nc.tensor.matmul(pfil[:], wup_t[:, 0:CH], wup_t[:],
                                     start=True, stop=True)
                dst = dst_t[:, a:b]
                e = ep_state[0] = (ep_state[0] + 1) % 3
                if e == 1:
                    if l < NLIN - 1:
                        nc.vector.tensor_scalar(dst, pz[:, :n],
                                                b_t[:, l:l + 1], 0.0,
                                                OP.add, OP.max)
                    else:
                        nc.vector.tensor_scalar_add(dst, pz[:, :n],
                                                    b_t[:, l:l + 1])
                else:
                    nc.scalar.activation(dst, pz[:, :n],
                                         AF.Relu if l < NLIN - 1 else AF.Identity,
                                         bias=b_t[:, l:l + 1])

        # K mlp
        mlpk_t = work.tile([CH, KW], bf16, tag="mlpa", name="mlpk")
        emit_layer(0, xkt_t, mlpk_t, wk_t, bk_t, KW)
        emit_layer(1, mlpk_t, a2k_t, wk_t, bk_t, KW)
        emit_layer(2, a2k_t, kt_t, wk_t, bk_t, KW)

        # kn: K natural per chunk (bias-free; bias folded into stc), two
        # chunks per PSUM tile so the drain copies halve
        if os.environ.get("KKNP", "0") != "0":
            for k0 in range(0, NKC, 2):
                pkn = pb.tile([CH, FMAX], f32, tag="pb", name="pkn")
                for k in (k0, k0 + 1):
                    if k >= NKC:
                        continue
                    s, p = sideof[k], kpos[k]
                    nc.tensor.matmul(pkn[0:CK, D * (k - k0):D * (k - k0 + 1)],
                                     a2k_t[64 * s:64 * s + 64, CH * p:CH * p + CK],
                                     wk_t[64 * s:64 * s + 64, 2 * D:3 * D],
                                     start=True, stop=True,
                                     tile_position=(64 * s, 0))
                n = D * min(2, NKC - k0)
                cyc_copy(kn_t[0:CK, D * k0:D * k0 + n], pkn[0:CK, :n])
        else:
            for k in range(NKC):
                s, p = sideof[k], kpos[k]
                pkn = pb.tile([CH, FMAX], f32, tag="pb", name="pkn")
                nc.tensor.matmul(pkn[0:CK, :D],
                                 a2k_t[64 * s:64 * s + 64, CH * p:CH * p + CK],
                                 wk_t[64 * s:64 * s + 64, 2 * D:3 * D],
                                 start=True, stop=True, tile_position=(64 * s, 0))
                cyc_copy(kn_t[0:CK, D * k:D * (k + 1)], pkn[0:CK, :D])

        # grouped S chunks: 4 matmuls of 8 chunks (diag blocks) + straggler
        psc = ps.tile([128, FMAX], f32, tag="ps", name="psc")
        for g in range(NG):
            nc.tensor.matmul(psc[32 * g:32 * g + 16, 0:FMAX],
                             v2_t[0:CK, 16 * g:16 * g + 16],
                             kn_t[0:CK, FMAX * g:FMAX * (g + 1)],
                             start=True, stop=True, tile_position=(0, 32 * g))
        psB = pb.tile([CH, FMAX], f32, tag="pb", name="psB")
        nc.tensor.matmul(psB[0:2, :D], v2_t[0:CK, 64:66],
                         kn_t[0:CK, D * 32:D * 33],
                         start=True, stop=True, tile_position=(0, 0))
        # stripe drain: PSUM group stripes -> SBUF bf16 (32-aligned partition
        # starts; finer PSUM reads are illegal). Gap rows were memset once so
        # the prefix matmuls' zero weights never hit NaN garbage.
        for g in range(NG):
            cyc_copy(sct1_t[32 * g:32 * g + 16, :], psc[32 * g:32 * g + 16, :])
        cyc_copy(scTB_t[:], psB[0:2, :D])

        # Q mlp layer 1 keeps the PE busy across the srun dependency chain
        mlpq_t = work.tile([CH, NW2], bf16, tag="mlpa", name="mlpq")
        emit_layer(0, qb_t, mlpq_t, wq_t, bq_t, NW2)

        # prefix sums via 9 accumulating triangular matmuls straight off the
        # stripes (lmatA_i's zero rows mask the off-diagonal stripe garbage),
        # then the host bias-prefix correction is added in the PSUM drain
        pl = pb.tile([CH, FMAX], f32, tag="pb", name="pl")
        for i in range(8):
            nc.tensor.matmul(pl[0:NR, :D], lA_t[:, NR * i:NR * (i + 1)],
                             sct1_t[:, D * i:D * (i + 1)],
                             start=(i == 0), stop=False, tile_position=(0, 0))
        nc.tensor.matmul(pl[0:NR, :D], lB_t[:], scTB_t[:], start=False, stop=True,
                         tile_position=(0, 0))
        nc.vector.tensor_add(srunT_t[:], pl[0:NR, :D], stc_t[:])

        # transpose back via the side-order permutation matmul, duplicated
        # into both halves; srun lands pre-sorted for one strided copy/side
        pt = pb.tile([CH, FMAX], f32, tag="pb", name="pt")
        nc.tensor.matmul(pt[0:64, 0:NR], srunT_t[:], p68_t[:],
                         start=True, stop=True, tile_position=(0, 0))
        nc.tensor.matmul(pt[64:128, 0:NR], srunT_t[:], p68_t[:],
                         start=True, stop=True, tile_position=(0, 64))
        # inject srun_k into every chunk's 2 reserved kt columns: one strided
        # copy per side (33 tiny copies serialized horribly here before)
        if os.environ.get("KINJ", "strided") == "strided":
            for s, nss, base in ((0, ns0, 0), (1, ns1, 2 * ns0)):
                dst = kt_t[64 * s:64 * s + 64, 0:KW].rearrange(
                    "p (c w) -> p c w", w=CH)[:, 0:nss, CK:CH]
                src = pt[64 * s:64 * s + 64, base:base + 2 * nss].rearrange(
                    "p (c two) -> p c two", two=2)
                if s == 0:
                    nc.scalar.copy(dst, src)
                else:
                    nc.vector.tensor_copy(dst, src)
        else:
            for k in range(NKC):
                s, p = sideof[k], kpos[k]
                j0 = (0 if s == 0 else 2 * ns0) + 2 * p
                cyc_copy(kt_t[64 * s:64 * s + 64, CH * p + CK:CH * p + CH],
                         pt[64 * s:64 * s + 64, j0:j0 + 2])
        # total state for the tails
        nc.scalar.copy(st_t[0:64, :], pt[0:64, 2 * NKC:2 * NKC + 2])
        nc.scalar.copy(st_t[64:128, :], pt[64:128, 2 * NKC:2 * NKC + 2])

        # Q mlp layers 2-3 (mid shares qt_t — Tile serializes per-block WAR)
        emit_layer(1, mlpq_t, qt_t, wq_t, bq_t, NW2)
        emit_layer(2, qt_t, qt_t, wq_t, bq_t, NW2)

        # tails (state-only), sides paired on PE quadrants
        for a in range(0, max(tlen), FMAX):
            for s in range(2):
                b = min(a + FMAX, tlen[s])
                if a >= b:
                    continue
                n = b - a
                po = pout.tile([CH, FMAX], f32, tag="pout", name="po_t")
                nc.tensor.matmul(po[64 * s:64 * s + 2, :n], st_t[64 * s:64 * s + 64, :],
                                 qt_t[64 * s:64 * s + 64, TOFF + a:TOFF + b],
                                 start=True, stop=True,
                                 tile_position=(64 * s, 64 * s))
                cyc_copy(outs_t[64 * s:64 * s + 2, TOFF + a:TOFF + b],
                         po[64 * s:64 * s + 2, :n])
        for s in range(2):
            if tlen[s] > 0:
                nc.sync.dma_start(d_out[2 * s:2 * s + 2, TOFF:TOFF + tlen[s]],
                                  outs_t[64 * s:64 * s + 2, TOFF:TOFF + tlen[s]])

        # band chunks: one masked matmul pair per chunk, software-pipelined
        # one deep so the PE never stalls on the DVE mask multiply
        segs = []
        for k in korder:
            nq = NB[k]
            s, p, o0 = sideof[k], kpos[k], int(qoff[k])
            for a in range(0, nq, FMAX):
                segs.append((k, s, p, o0 + a, min(a + FMAX, nq) - a))
        pBs = [None] * len(segs)

        def emit_pB(i):
            k, s, p, o, n = segs[i]
            pB = pBs[i] = pb.tile([CH, FMAX], f32, tag="pb", name="pB")
            nc.tensor.matmul(pB[:, :n], kt_t[64 * s:64 * s + 64, CH * p:CH * (p + 1)],
                             qt_t[64 * s:64 * s + 64, o:o + n],
                             start=True, stop=True, tile_position=(64 * s, 0))

        def emit_rest(i):
            k, s, p, o, n = segs[i]
            bm = work.tile([CH, FMAX], bf16, tag="bm", name="bm")
            nc.vector.tensor_mul(bm[:, :n], pBs[i][:, :n],
                                 msk_t[:, s * TOFF + o:s * TOFF + o + n])
            po = pout.tile([CH, FMAX], f32, tag="pout", name="po")
            nc.tensor.matmul(po[64 * s:64 * s + 2, :n], v2_t[:, 2 * k:2 * k + 2],
                             bm[:, :n], start=True, stop=True,
                             tile_position=(0, 64 * s))
            cyc_copy(outs_t[64 * s:64 * s + 2, o:o + n],
                     po[64 * s:64 * s + 2, :n])

        if os.environ.get("KSEQ", "0") != "0":
            for i in range(len(segs)):
                emit_pB(i)
                emit_rest(i)
        elif segs:
            emit_pB(0)
            for i in range(1, len(segs)):
                emit_pB(i)
                emit_rest(i - 1)
            emit_rest(len(segs) - 1)
        for s in range(2):
            w = plan["lb"] if s == 0 else plan["rb"]
            if w > 0:
                nc.sync.dma_start(d_out[2 * s:2 * s + 2, 0:w],
                                  outs_t[64 * s:64 * s + 2, 0:w])

    nc.finalize()
    return nc


_CACHE = {}


def kernel(X, wq_w, wq_b, wk_w, wk_b):
    from concourse.bass_utils import run_bass_kernel_spmd

    plan = make_plan(X)
    in_maps = make_inputs(X, wq_w, wq_b, wk_w, wk_b, plan)
    key = (tuple(plan["NB"]), tuple(plan["sideof"]), plan["TOFF"],
           tuple(plan["tlen"]), plan["NW2"])
    if key not in _CACHE:
        _CACHE[key] = build_graph(plan)
    nc = _CACHE[key]
    res = run_bass_kernel_spmd(nc, in_maps, core_ids=list(range(NCORES)),
                               trace=bool(int(os.environ.get("KTRACE", "0"))))
    outs = [res.results[c]["out"] for c in range(NCORES)]
    y = scatter_outputs(plan, outs)
    if os.environ.get("KTRACE", "0") != "0":
        kernel.last_result = res
    return y[None]  # (1, T, 2)
